# revision 9
# baseline (speedup 1.0000x reference)
"""GCN (3x GCNConv + BN + residual, mean-pool, MLP head) on 8 trn2 NeuronCores.

Sharding: nodes are assigned to 392 blocks of 128 via degree-balanced snake
packing (equalizes per-block incident-edge counts, minimizing gather-chunk
padding); 49 blocks per core. Each core owns the edges whose TARGET lands in
its blocks (plus self-loops). GCN normalization is linear, so each layer
aggregates raw input features over incident edges (one indirect-DMA gather of
128 source rows + one PE matmul with a selection matrix per 128-edge chunk),
then applies the folded linear+BN epilogue. Activation tables are bf16;
AllGathers between layers are chunked (7 groups of 7 blocks) so they overlap
the producing layer's compute. Per-graph pooled sums are AllReduced; the tiny
MLP head runs redundantly on every core.

Device kernel per (layer, target-block of 128 nodes):
  for each 128-edge chunk: indirect-DMA gather of source rows (bf16);
  one fused DVE pass builds all selection matrices S[e,t] = w'[e]*(tl[e]==t);
  PE: aggT[f,t] += xr[:,f-chunk].T @ S_j (PSUM, bf16 inputs);
  hT[o,t] = sum_f W'[f,o].T @ agg[f,t]; ACT relu + folded bias; DVE +tsh
  (+residual); PE transpose back to [t,o] bf16 rows for the next layer's
  gather table / pooling.
"""
import math
import os
import sys

import numpy as np

sys.path.insert(0, "/opt/trn_rl_repo")

N_NODES = 50000
N_EDGES = 800000
IN_DIM = 128
HID = 256
OUT_DIM = 1
N_GRAPHS = 512
BN_EPS = 1e-5
NCORES = 8
P = 128
NBLK = 49                            # blocks per core
NTOT = NBLK * NCORES                 # 392 blocks globally
PADN = NBLK * P                      # 6272 rows per core (incl pad slots)
XROWS = PADN * NCORES                # 50176 rows in allgathered tables
AGRP = 7                             # blocks per chunked-AllGather group
NGRP = NBLK // AGRP                  # 7 groups


def _build_program(chunks):
    from concourse import bass, bacc, mybir, tile
    from concourse.masks import make_identity

    f32 = mybir.dt.float32
    bf16 = mybir.dt.bfloat16
    i32 = mybir.dt.int32
    AF = mybir.ActivationFunctionType
    OP = mybir.AluOpType

    C = chunks
    GRows = AGRP * P                 # rows per AllGather group (896)

    nc = bacc.Bacc("TRN2", target_bir_lowering=False, debug=False,
                   num_devices=NCORES)

    x_in = nc.declare_dram_parameter("x", [N_NODES, IN_DIM], bf16, isOutput=False)
    idx1 = nc.declare_dram_parameter("idx1", [P, NBLK * C], i32, isOutput=False)
    idx2 = nc.declare_dram_parameter("idx2", [P, NBLK * C], i32, isOutput=False)
    meta = nc.declare_dram_parameter("meta", [P, NBLK * 2 * C], bf16, isOutput=False)
    bcol = nc.declare_dram_parameter("bcol", [P, NBLK], f32, isOutput=False)
    w1p = nc.declare_dram_parameter("w1p", [IN_DIM, HID], bf16, isOutput=False)
    w2p = nc.declare_dram_parameter("w2p", [HID, HID], bf16, isOutput=False)
    w3p = nc.declare_dram_parameter("w3p", [HID, HID], bf16, isOutput=False)
    bias = nc.declare_dram_parameter("bias", [P, 6], f32, isOutput=False)
    tsh = nc.declare_dram_parameter("tsh", [P, 6], f32, isOutput=False)
    lw1 = nc.declare_dram_parameter("lw1", [HID, HID], f32, isOutput=False)
    lb1c = nc.declare_dram_parameter("lb1c", [P, 2], f32, isOutput=False)
    lw2 = nc.declare_dram_parameter("lw2", [P, 2], f32, isOutput=False)
    lb2c = nc.declare_dram_parameter("lb2c", [1, 1], f32, isOutput=False)
    icnt = nc.declare_dram_parameter("icnt", [P, N_GRAPHS], f32, isOutput=False)
    out = nc.declare_dram_parameter("out", [1, N_GRAPHS], f32, isOutput=True)

    with tile.TileContext(nc) as tc:
        with tc.tile_pool(name="const", bufs=1) as cpool, \
             tc.tile_pool(name="rows", bufs=8) as rpool, \
             tc.tile_pool(name="smat", bufs=2) as spool, \
             tc.tile_pool(name="work", bufs=2) as wpool, \
             tc.tile_pool(name="resid", bufs=1) as residp, \
             tc.tile_pool(name="hrow", bufs=3) as hpool, \
             tc.tile_pool(name="psum", bufs=2, space="PSUM") as ppool, \
             tc.tile_pool(name="psump", bufs=1, space="PSUM") as ppoolp, \
             tc.tile_pool(name="dram", bufs=1, space="DRAM") as dpool:

            iota_i = cpool.tile([P, P], i32, tag="ioi")
            nc.gpsimd.iota(iota_i[:], pattern=[[1, P]], base=0, channel_multiplier=0)
            iota_b = cpool.tile([P, P], bf16, tag="iob")
            nc.vector.tensor_copy(iota_b[:], iota_i[:])
            iota5_i = cpool.tile([P, N_GRAPHS], i32, tag="io5i")
            nc.gpsimd.iota(iota5_i[:], pattern=[[1, N_GRAPHS]], base=0, channel_multiplier=0)
            iota5_f = cpool.tile([P, N_GRAPHS], f32, tag="io5f")
            nc.vector.tensor_copy(iota5_f[:], iota5_i[:])
            ident = cpool.tile([P, P], bf16, tag="ident")
            make_identity(nc, ident[:])

            bias_t = cpool.tile([P, 6], f32, tag="bias")
            nc.sync.dma_start(out=bias_t[:], in_=bias[:, :])
            tsh_t = cpool.tile([P, 6], f32, tag="tsh")
            nc.sync.dma_start(out=tsh_t[:], in_=tsh[:, :])

            w1_t = cpool.tile([IN_DIM, HID], bf16, tag="w1")
            nc.sync.dma_start(out=w1_t[:], in_=w1p[:, :])
            w2_t = [cpool.tile([P, HID], bf16, tag=f"w2_{k}", name=f"w2_{k}") for k in range(2)]
            w3_t = [cpool.tile([P, HID], bf16, tag=f"w3_{k}", name=f"w3_{k}") for k in range(2)]
            for k in range(2):
                nc.sync.dma_start(out=w2_t[k][:], in_=w2p[k * P:(k + 1) * P, :])
                nc.sync.dma_start(out=w3_t[k][:], in_=w3p[k * P:(k + 1) * P, :])

            # all per-block metadata loaded upfront (small)
            idx1_t = cpool.tile([P, NBLK * C], i32, tag="idx1")
            nc.sync.dma_start(out=idx1_t[:], in_=idx1[:, :])
            idx2_t = cpool.tile([P, NBLK * C], i32, tag="idx2")
            nc.sync.dma_start(out=idx2_t[:], in_=idx2[:, :])
            meta_t = cpool.tile([P, NBLK * 2 * C], bf16, tag="meta")
            nc.sync.dma_start(out=meta_t[:], in_=meta[:, :])
            bcol_t = cpool.tile([P, NBLK], f32, tag="bcol")
            nc.sync.dma_start(out=bcol_t[:], in_=bcol[:, :])
            icnt_t = cpool.tile([P, N_GRAPHS], f32, tag="icnt")
            nc.sync.dma_start(out=icnt_t[:], in_=icnt[:, :])

            hloc1 = [dpool.tile([GRows, HID], bf16, tag=f"hloc1_{g}",
                                 name=f"hloc1_{g}") for g in range(NGRP)]
            hloc2 = [dpool.tile([GRows, HID], bf16, tag=f"hloc2_{g}",
                                 name=f"hloc2_{g}") for g in range(NGRP)]
            xnext1 = dpool.tile([NGRP, NCORES, GRows, HID], bf16, tag="xn1")
            xnext2 = dpool.tile([NGRP, NCORES, GRows, HID], bf16, tag="xn2")
            prdram = dpool.tile([HID, N_GRAPHS], f32, tag="prd")
            ardram = dpool.tile([HID, N_GRAPHS], f32, tag="ard")

            resid = [[residp.tile([P, P], bf16, tag=f"r{b}h{h}", name=f"r{b}h{h}")
                      for h in range(2)] for b in range(NBLK)]

            pooled_ps = [ppoolp.tile([P, N_GRAPHS], f32, tag=f"pool{h}", name=f"pool{h}")
                         for h in range(2)]

            def build_smat(b):
                """One fused DVE pass per block: S[e, j*128+t] = w'[j] * (tl[j]==t)."""
                s01 = spool.tile([P, C * P], bf16, tag="s01")
                smat = spool.tile([P, C * P], bf16, tag="smat")
                tl_ap = meta_t[:, b * 2 * C: b * 2 * C + C].unsqueeze(2) \
                    .broadcast_to([P, C, P])
                w_ap = meta_t[:, b * 2 * C + C: (b + 1) * 2 * C].unsqueeze(2) \
                    .broadcast_to([P, C, P])
                io_ap = iota_b[:, :].unsqueeze(1).broadcast_to([P, C, P])
                s01_3d = s01[:].rearrange("p (c t) -> p c t", c=C)
                smat_3d = smat[:].rearrange("p (c t) -> p c t", c=C)
                nc.vector.tensor_tensor(out=s01_3d, in0=tl_ap, in1=io_ap,
                                        op=mybir.AluOpType.is_equal)
                nc.vector.tensor_tensor(out=smat_3d, in0=s01_3d, in1=w_ap,
                                        op=mybir.AluOpType.mult)
                return smat

            def ag_one(hloc, xnext, g):
                nc.gpsimd.collective_compute(
                    "AllGather", bass.mybir.AluOpType.bypass,
                    replica_groups=[list(range(NCORES))],
                    ins=[hloc[g][:, :]], outs=[xnext[g]])

            def layer(li, tab, fdim, idx_all, wtiles, bc0, hloc, xnext=None):
                nf = fdim // P
                for b in range(NBLK):
                    if hloc is not None and b % AGRP == 0:
                        gg = b // AGRP - 2
                        if gg >= 0:
                            ag_one(hloc, xnext, gg)
                    xr = rpool.tile([P, C * fdim], bf16, tag="xr")
                    for j in range(C):
                        nc.gpsimd.indirect_dma_start(
                            out=xr[:, j * fdim:(j + 1) * fdim], out_offset=None,
                            in_=tab,
                            in_offset=bass.IndirectOffsetOnAxis(
                                ap=idx_all[:, b * C + j:b * C + j + 1], axis=0),
                        )
                    smat = build_smat(b)

                    aggT = [ppool.tile([P, P], f32, tag=f"agg{k}", name=f"aggps{k}")
                            for k in range(nf)]
                    for j in range(C):
                        for k in range(nf):
                            nc.tensor.matmul(
                                aggT[k][:],
                                lhsT=xr[:, j * fdim + k * P: j * fdim + (k + 1) * P],
                                rhs=smat[:, j * P:(j + 1) * P],
                                start=(j == 0), stop=(j == C - 1))

                    aggs = [wpool.tile([P, P], bf16, tag=f"aggs{k}", name=f"aggs{k}")
                            for k in range(nf)]
                    for k in range(nf):
                        nc.scalar.copy(aggs[k][:], aggT[k][:])

                    hrow = hpool.tile([P, HID], bf16, tag="hrow")
                    for h in range(2):
                        hT_ps = ppool.tile([P, P], f32, tag="ht")
                        for k in range(nf):
                            nc.tensor.matmul(
                                hT_ps[:], lhsT=wtiles[k][:, h * P:(h + 1) * P],
                                rhs=aggs[k][:], start=(k == 0), stop=(k == nf - 1))
                        hTs = wpool.tile([P, P], f32, tag=f"hTs{h}")
                        nc.scalar.activation(hTs[:], hT_ps[:], AF.Relu,
                                             bias=bias_t[:, bc0 + h:bc0 + h + 1])
                        if li == 0:
                            nc.vector.tensor_scalar(
                                out=resid[b][h][:], in0=hTs[:],
                                scalar1=tsh_t[:, bc0 + h:bc0 + h + 1], scalar2=None,
                                op0=OP.add)
                        else:
                            u = wpool.tile([P, P], bf16, tag=f"u{h}")
                            nc.vector.tensor_scalar(
                                out=u[:], in0=hTs[:],
                                scalar1=tsh_t[:, bc0 + h:bc0 + h + 1], scalar2=None,
                                op0=OP.add)
                            nc.vector.tensor_tensor(
                                out=resid[b][h][:], in0=resid[b][h][:], in1=u[:],
                                op=OP.add)
                        tp_ps = ppool.tile([P, P], bf16, tag="ht")
                        nc.tensor.transpose(tp_ps[:], resid[b][h][:], ident[:])
                        nc.scalar.copy(hrow[:, h * P:(h + 1) * P], tp_ps[:])

                    if hloc is not None:
                        nc.sync.dma_start(
                            out=hloc[b // AGRP][(b % AGRP) * P:(b % AGRP + 1) * P, :],
                            in_=hrow[:])
                    else:
                        # L3: pool inline. mblk[t, g] = (batch[t]==g)
                        mblk = spool.tile([P, N_GRAPHS], bf16, tag="mblk")
                        nc.vector.tensor_tensor(
                            out=mblk[:],
                            in0=bcol_t[:, b:b + 1].broadcast_to([P, N_GRAPHS]),
                            in1=iota5_f[:], op=OP.is_equal)
                        for h in range(2):
                            nc.tensor.matmul(
                                pooled_ps[h][:], lhsT=hrow[:, h * P:(h + 1) * P],
                                rhs=mblk[:], start=(b == 0), stop=(b == NBLK - 1))

            def allgather_tail(hloc, xnext):
                for g in (NGRP - 2, NGRP - 1):
                    ag_one(hloc, xnext, g)

            tab1 = x_in[:, :]
            layer(0, tab1, IN_DIM, idx1_t, [w1_t], 0, hloc1, xnext1)
            allgather_tail(hloc1, xnext1)
            tab2 = xnext1[:, :, :, :].rearrange("g c r f -> (g c r) f")
            layer(1, tab2, HID, idx2_t, w2_t, 2, hloc2, xnext2)
            allgather_tail(hloc2, xnext2)
            tab3 = xnext2[:, :, :, :].rearrange("g c r f -> (g c r) f")
            layer(2, tab3, HID, idx2_t, w3_t, 4, None)

            # pooled partial sums -> DRAM -> AllReduce
            for h in range(2):
                ps = wpool.tile([P, N_GRAPHS], f32, tag=f"poolsb{h}")
                nc.vector.tensor_copy(ps[:], pooled_ps[h][:])
                nc.sync.dma_start(out=prdram[h * P:(h + 1) * P, :], in_=ps[:])
            nc.gpsimd.collective_compute(
                "AllReduce", bass.mybir.AluOpType.add,
                replica_groups=[list(range(NCORES))],
                ins=[prdram[:, :]], outs=[ardram[:, :]])

            # head: h1T[o,g] = relu(lw1.T @ (pooledT*icnt) + lb1); out = lw2.T @ h1T + lb2
            lw1_t = [cpool.tile([P, HID], f32, tag=f"lw1_{k}", name=f"lw1_{k}") for k in range(2)]
            lw2_t = cpool.tile([P, 2], f32, tag="lw2")
            lb1_t = cpool.tile([P, 2], f32, tag="lb1")
            lb2_t = cpool.tile([1, 1], f32, tag="lb2")
            for k in range(2):
                nc.sync.dma_start(out=lw1_t[k][:], in_=lw1[k * P:(k + 1) * P, :])
            nc.sync.dma_start(out=lw2_t[:], in_=lw2[:, :])
            nc.sync.dma_start(out=lb1_t[:], in_=lb1c[:, :])
            nc.sync.dma_start(out=lb2_t[:], in_=lb2c[:, :])

            par = []
            for k in range(2):
                pk = wpool.tile([P, N_GRAPHS], f32, tag=f"par{k}")
                nc.sync.dma_start(out=pk[:], in_=ardram[k * P:(k + 1) * P, :])
                pks = wpool.tile([P, N_GRAPHS], f32, tag=f"pars{k}")
                nc.vector.tensor_tensor(out=pks[:], in0=pk[:], in1=icnt_t[:], op=OP.mult)
                par.append(pks)
            h1s = []
            for h in range(2):
                h1_ps = ppool.tile([P, N_GRAPHS], f32, tag="agg0")
                for k in range(2):
                    nc.tensor.matmul(h1_ps[:], lhsT=lw1_t[k][:, h * P:(h + 1) * P],
                                     rhs=par[k][:], start=(k == 0), stop=(k == 1))
                h1sb = wpool.tile([P, N_GRAPHS], f32, tag=f"h1s{h}")
                nc.scalar.activation(h1sb[:], h1_ps[:], AF.Relu,
                                     bias=lb1_t[:, h:h + 1])
                h1s.append(h1sb)
            out_ps = ppool.tile([1, N_GRAPHS], f32, tag="agg1")
            for h in range(2):
                nc.tensor.matmul(out_ps[:], lhsT=lw2_t[:, h:h + 1],
                                 rhs=h1s[h][:], start=(h == 0), stop=(h == 1))
            out_sb = wpool.tile([1, N_GRAPHS], f32, tag="outs")
            nc.vector.tensor_scalar(out=out_sb[:], in0=out_ps[:],
                                    scalar1=lb2_t[0:1, 0:1], scalar2=None, op0=OP.add)
            nc.sync.dma_start(out=out[:, :], in_=out_sb[:])

    nc.compile()
    return nc


def _preprocess(edge_index, batch):
    """Degree-balanced node->block assignment + per-core edge lists grouped by
    target block, padded to uniform chunks."""
    src = np.asarray(edge_index[0], dtype=np.int64)
    tgt = np.asarray(edge_index[1], dtype=np.int64)
    batch = np.asarray(batch, dtype=np.int64)

    indeg = np.bincount(tgt, minlength=N_NODES).astype(np.int64)
    deg = indeg.astype(np.float64) + 1.0
    dinv = 1.0 / np.sqrt(deg)

    # balanced snake assignment of 50176 slots (incl 176 weight-0 virtual)
    slots = XROWS
    wts = np.concatenate([indeg + 1, np.zeros(slots - N_NODES, np.int64)])
    order = np.argsort(-wts, kind="stable")
    assign_block = np.empty(slots, np.int64)
    fwd = np.arange(NTOT)
    for r in range(P):
        seg = order[r * NTOT:(r + 1) * NTOT]
        assign_block[seg] = fwd if r % 2 == 0 else fwd[::-1]
    perm = np.argsort(assign_block, kind="stable")
    newpos = np.empty(slots, np.int64)
    newpos[perm] = np.arange(slots)

    allsrc = np.concatenate([src, np.arange(N_NODES, dtype=np.int64)])
    alltgt = np.concatenate([tgt, np.arange(N_NODES, dtype=np.int64)])
    allw = (dinv[allsrc] * dinv[alltgt]).astype(np.float32)

    tgt_np = newpos[alltgt]
    blkkey = tgt_np // P
    order_e = np.argsort(blkkey, kind="stable")
    allsrc, alltgt, allw = allsrc[order_e], alltgt[order_e], allw[order_e]
    tgt_np = tgt_np[order_e]

    counts = np.bincount(blkkey, minlength=NTOT)
    chunks = int(math.ceil(counts.max() / P))
    C = chunks

    GR = AGRP * P
    blk_start = np.zeros(NTOT + 1, dtype=np.int64)
    np.cumsum(counts, out=blk_start[1:])

    import ml_dtypes
    per_core = []
    for c in range(NCORES):
        idx1 = np.zeros((NBLK, P, C), dtype=np.int32)
        idx2 = np.zeros((NBLK, P, C), dtype=np.int32)
        meta = np.zeros((NBLK, P, 2 * C), dtype=np.float32)
        for b in range(NBLK):
            g = c * NBLK + b
            lo, hi = blk_start[g], blk_start[g + 1]
            n = hi - lo
            s1 = allsrc[lo:hi].astype(np.int32)
            np_src = newpos[allsrc[lo:hi]]
            core_s = np_src // PADN
            r_s = np_src % PADN
            s2 = ((r_s // GR) * (NCORES * GR) + core_s * GR + (r_s % GR)
                  ).astype(np.int32)
            tl = (tgt_np[lo:hi] - g * P).astype(np.float32)
            ww = allw[lo:hi]
            npad = C * P - n
            if npad:
                s1 = np.pad(s1, (0, npad))
                s2 = np.pad(s2, (0, npad))
                tl = np.pad(tl, (0, npad))
                ww = np.pad(ww, (0, npad))
            idx1[b] = s1.reshape(C, P).T
            idx2[b] = s2.reshape(C, P).T
            meta[b, :, :C] = tl.reshape(C, P).T
            meta[b, :, C:] = ww.reshape(C, P).T
        # batch column for pooling (pad/virtual rows -> -1)
        core_slots = perm[c * PADN:(c + 1) * PADN]   # orig ids in new order
        bvals = np.where(core_slots < N_NODES,
                         batch[np.minimum(core_slots, N_NODES - 1)], -1.0)
        bcol = bvals.reshape(NBLK, P).T.astype(np.float32)  # [P, NBLK]
        per_core.append(dict(
            idx1=idx1.transpose(1, 0, 2).reshape(P, NBLK * C).copy(),
            idx2=idx2.transpose(1, 0, 2).reshape(P, NBLK * C).copy(),
            meta=meta.transpose(1, 0, 2).reshape(P, NBLK * 2 * C)
                .astype(ml_dtypes.bfloat16),
            bcol=bcol.copy(),
        ))
    return per_core, chunks


def kernel(**inputs):
    import ml_dtypes
    from concourse.bass_utils import run_bass_kernel_spmd

    x = np.asarray(inputs["x"], dtype=np.float32)
    edge_index = np.asarray(inputs["edge_index"])
    batch = np.asarray(inputs["batch"])

    per_core, chunks = _preprocess(edge_index, batch)

    def g(k):
        return np.asarray(inputs[k], dtype=np.float32)

    params = {}
    Ws = [g("W1"), g("W2"), g("W3")]
    bs = [g("b1"), g("b2"), g("b3")]
    bias = np.zeros((P, 6), np.float32)
    tshv = np.zeros((P, 6), np.float32)
    wp = []
    for i in range(3):
        gam, be, m, v = g(f"g{i+1}"), g(f"be{i+1}"), g(f"m{i+1}"), g(f"v{i+1}")
        s = gam / np.sqrt(v + BN_EPS)
        assert (s > 0).all(), "BN scale must be positive for relu folding"
        wp.append((Ws[i] * s[None, :]).astype(ml_dtypes.bfloat16))
        bp = (bs[i] * s).astype(np.float32)
        tv = (be - m * s).astype(np.float32)
        bias[:, 2 * i] = bp[:P]
        bias[:, 2 * i + 1] = bp[P:]
        tshv[:, 2 * i] = tv[:P]
        tshv[:, 2 * i + 1] = tv[P:]
    params["w1p"], params["w2p"], params["w3p"] = wp
    params["bias"] = bias
    params["tsh"] = tshv
    params["lw1"] = g("lw1")
    lb1 = g("lb1")
    lb1c = np.zeros((P, 2), np.float32)
    lb1c[:, 0] = lb1[:P]
    lb1c[:, 1] = lb1[P:]
    params["lb1c"] = lb1c
    lw2v = g("lw2").reshape(HID)
    params["lw2"] = np.stack([lw2v[:P], lw2v[P:]], axis=1).copy()
    params["lb2c"] = g("lb2").reshape(1, 1).astype(np.float32)
    cnt = np.bincount(np.asarray(batch, dtype=np.int64), minlength=N_GRAPHS)
    icnt = (1.0 / np.maximum(cnt, 1)).astype(np.float32)
    params["icnt"] = np.tile(icnt[None, :], (P, 1))
    x_bf = x.astype(ml_dtypes.bfloat16)

    nc = _build_program(chunks)

    in_maps = []
    for c in range(NCORES):
        m = dict(params)
        m["x"] = x_bf
        m.update(per_core[c])
        in_maps.append(m)

    res = run_bass_kernel_spmd(nc, in_maps, list(range(NCORES)),
                               trace=bool(os.environ.get("GNN_TRACE")))
    if os.environ.get("GNN_TRACE"):
        print("HW exec time:", res.exec_time_ns, "ns")
    global _last_results
    _last_results = res.results
    o = res.results[0]["out"]
    return np.asarray(o, dtype=np.float32).reshape(N_GRAPHS, OUT_DIM)


# revision 10
# speedup vs baseline: 1.0801x; 1.0801x over previous
"""GCN (3x GCNConv + BN + residual, mean-pool, MLP head) on 8 trn2 NeuronCores.

Sharding: nodes are assigned to 392 blocks of 128 via degree-balanced snake
packing (equalizes per-block incident-edge counts, minimizing gather-chunk
padding); 49 blocks per core. Each core owns the edges whose TARGET lands in
its blocks (plus self-loops). GCN normalization is linear, so each layer
aggregates raw input features over incident edges (one indirect-DMA gather of
128 source rows + one PE matmul with a selection matrix per 128-edge chunk),
then applies the folded linear+BN epilogue. Activation tables are bf16;
AllGathers between layers are chunked (7 groups of 7 blocks) so they overlap
the producing layer's compute. Per-graph pooled sums are AllReduced; the tiny
MLP head runs redundantly on every core.

Device kernel per (layer, target-block of 128 nodes):
  for each 128-edge chunk: indirect-DMA gather of source rows (bf16);
  one fused DVE pass builds all selection matrices S[e,t] = w'[e]*(tl[e]==t);
  PE: aggT[f,t] += xr[:,f-chunk].T @ S_j (PSUM, bf16 inputs);
  hT[o,t] = sum_f W'[f,o].T @ agg[f,t]; ACT relu + folded bias; DVE +tsh
  (+residual); PE transpose back to [t,o] bf16 rows for the next layer's
  gather table / pooling.
"""
import math
import os
import sys

import numpy as np

sys.path.insert(0, "/opt/trn_rl_repo")

N_NODES = 50000
N_EDGES = 800000
IN_DIM = 128
HID = 256
OUT_DIM = 1
N_GRAPHS = 512
BN_EPS = 1e-5
NCORES = 8
P = 128
NBLK = 49                            # blocks per core
NTOT = NBLK * NCORES                 # 392 blocks globally
PADN = NBLK * P                      # 6272 rows per core (incl pad slots)
XROWS = PADN * NCORES                # 50176 rows in allgathered tables
AGRP = 7                             # blocks per chunked-AllGather group
NGRP = NBLK // AGRP                  # 7 groups


def _build_program(chunks):
    from concourse import bass, bacc, mybir, tile
    from concourse.masks import make_identity

    f32 = mybir.dt.float32
    bf16 = mybir.dt.bfloat16
    i32 = mybir.dt.int32
    AF = mybir.ActivationFunctionType
    OP = mybir.AluOpType

    C = chunks
    GRows = AGRP * P                 # rows per AllGather group (896)

    nc = bacc.Bacc("TRN2", target_bir_lowering=False, debug=False,
                   num_devices=NCORES)

    x_in = nc.declare_dram_parameter("x", [N_NODES, IN_DIM], bf16, isOutput=False)
    idx1 = nc.declare_dram_parameter("idx1", [P, NBLK * C], i32, isOutput=False)
    idx2 = nc.declare_dram_parameter("idx2", [P, NBLK * C], i32, isOutput=False)
    meta = nc.declare_dram_parameter("meta", [P, NBLK * 2 * C], bf16, isOutput=False)
    bcol = nc.declare_dram_parameter("bcol", [P, NBLK], f32, isOutput=False)
    w1p = nc.declare_dram_parameter("w1p", [IN_DIM, HID], bf16, isOutput=False)
    w2p = nc.declare_dram_parameter("w2p", [HID, HID], bf16, isOutput=False)
    w3p = nc.declare_dram_parameter("w3p", [HID, HID], bf16, isOutput=False)
    bias = nc.declare_dram_parameter("bias", [P, 6], f32, isOutput=False)
    tsh = nc.declare_dram_parameter("tsh", [P, 6], f32, isOutput=False)
    lw1 = nc.declare_dram_parameter("lw1", [HID, HID], f32, isOutput=False)
    lb1c = nc.declare_dram_parameter("lb1c", [P, 2], f32, isOutput=False)
    lw2 = nc.declare_dram_parameter("lw2", [P, 2], f32, isOutput=False)
    lb2c = nc.declare_dram_parameter("lb2c", [1, 1], f32, isOutput=False)
    icnt = nc.declare_dram_parameter("icnt", [P, N_GRAPHS], f32, isOutput=False)
    out = nc.declare_dram_parameter("out", [1, N_GRAPHS], f32, isOutput=True)

    with tile.TileContext(nc) as tc:
        with tc.tile_pool(name="const", bufs=1) as cpool, \
             tc.tile_pool(name="rows", bufs=8) as rpool, \
             tc.tile_pool(name="smat", bufs=2) as spool, \
             tc.tile_pool(name="work", bufs=2) as wpool, \
             tc.tile_pool(name="resid", bufs=1) as residp, \
             tc.tile_pool(name="hrow", bufs=3) as hpool, \
             tc.tile_pool(name="psum", bufs=2, space="PSUM") as ppool, \
             tc.tile_pool(name="psump", bufs=1, space="PSUM") as ppoolp, \
             tc.tile_pool(name="dram", bufs=1, space="DRAM") as dpool:

            iota_i = cpool.tile([P, P], i32, tag="ioi")
            nc.gpsimd.iota(iota_i[:], pattern=[[1, P]], base=0, channel_multiplier=0)
            iota_b = cpool.tile([P, P], bf16, tag="iob")
            nc.vector.tensor_copy(iota_b[:], iota_i[:])
            iota5_i = cpool.tile([P, N_GRAPHS], i32, tag="io5i")
            nc.gpsimd.iota(iota5_i[:], pattern=[[1, N_GRAPHS]], base=0, channel_multiplier=0)
            iota5_f = cpool.tile([P, N_GRAPHS], f32, tag="io5f")
            nc.vector.tensor_copy(iota5_f[:], iota5_i[:])
            ident = cpool.tile([P, P], bf16, tag="ident")
            make_identity(nc, ident[:])

            bias_t = cpool.tile([P, 6], f32, tag="bias")
            nc.sync.dma_start(out=bias_t[:], in_=bias[:, :])
            tsh_t = cpool.tile([P, 6], f32, tag="tsh")
            nc.sync.dma_start(out=tsh_t[:], in_=tsh[:, :])

            w1_t = cpool.tile([IN_DIM, HID], bf16, tag="w1")
            nc.sync.dma_start(out=w1_t[:], in_=w1p[:, :])
            w2_t = [cpool.tile([P, HID], bf16, tag=f"w2_{k}", name=f"w2_{k}") for k in range(2)]
            w3_t = [cpool.tile([P, HID], bf16, tag=f"w3_{k}", name=f"w3_{k}") for k in range(2)]
            for k in range(2):
                nc.sync.dma_start(out=w2_t[k][:], in_=w2p[k * P:(k + 1) * P, :])
                nc.sync.dma_start(out=w3_t[k][:], in_=w3p[k * P:(k + 1) * P, :])

            # all per-block metadata loaded upfront (small)
            idx1_t = cpool.tile([P, NBLK * C], i32, tag="idx1")
            nc.sync.dma_start(out=idx1_t[:], in_=idx1[:, :])
            idx2_t = cpool.tile([P, NBLK * C], i32, tag="idx2")
            nc.sync.dma_start(out=idx2_t[:], in_=idx2[:, :])
            meta_t = cpool.tile([P, NBLK * 2 * C], bf16, tag="meta")
            nc.sync.dma_start(out=meta_t[:], in_=meta[:, :])
            bcol_t = cpool.tile([P, NBLK], f32, tag="bcol")
            nc.sync.dma_start(out=bcol_t[:], in_=bcol[:, :])
            icnt_t = cpool.tile([P, N_GRAPHS], f32, tag="icnt")
            nc.sync.dma_start(out=icnt_t[:], in_=icnt[:, :])

            hloc1 = [dpool.tile([GRows, HID], bf16, tag=f"hloc1_{g}",
                                 name=f"hloc1_{g}") for g in range(NGRP)]
            hloc2 = [dpool.tile([GRows, HID], bf16, tag=f"hloc2_{g}",
                                 name=f"hloc2_{g}") for g in range(NGRP)]
            xnext1 = dpool.tile([NGRP, NCORES, GRows, HID], bf16, tag="xn1")
            xnext2 = dpool.tile([NGRP, NCORES, GRows, HID], bf16, tag="xn2")
            prdram = dpool.tile([HID, N_GRAPHS], f32, tag="prd")
            ardram = dpool.tile([HID, N_GRAPHS], f32, tag="ard")

            resid = [[residp.tile([P, P], bf16, tag=f"r{b}h{h}", name=f"r{b}h{h}")
                      for h in range(2)] for b in range(NBLK)]

            pooled_ps = [ppoolp.tile([P, N_GRAPHS], f32, tag=f"pool{h}", name=f"pool{h}")
                         for h in range(2)]

            def build_smat(b):
                """One fused DVE pass per block: S[e, j*128+t] = w'[j] * (tl[j]==t)."""
                s01 = spool.tile([P, C * P], bf16, tag="s01")
                smat = spool.tile([P, C * P], bf16, tag="smat")
                tl_ap = meta_t[:, b * 2 * C: b * 2 * C + C].unsqueeze(2) \
                    .broadcast_to([P, C, P])
                w_ap = meta_t[:, b * 2 * C + C: (b + 1) * 2 * C].unsqueeze(2) \
                    .broadcast_to([P, C, P])
                io_ap = iota_b[:, :].unsqueeze(1).broadcast_to([P, C, P])
                s01_3d = s01[:].rearrange("p (c t) -> p c t", c=C)
                smat_3d = smat[:].rearrange("p (c t) -> p c t", c=C)
                nc.vector.tensor_tensor(out=s01_3d, in0=tl_ap, in1=io_ap,
                                        op=mybir.AluOpType.is_equal)
                nc.vector.tensor_tensor(out=smat_3d, in0=s01_3d, in1=w_ap,
                                        op=mybir.AluOpType.mult)
                return smat

            def ag_one(hloc, xnext, g):
                nc.gpsimd.collective_compute(
                    "AllGather", bass.mybir.AluOpType.bypass,
                    replica_groups=[list(range(NCORES))],
                    ins=[hloc[g][:, :]], outs=[xnext[g]])

            def layer(li, tab, fdim, idx_all, wtiles, bc0, hloc, xnext=None):
                nf = fdim // P
                for b in range(NBLK):
                    xr = rpool.tile([P, C * fdim], bf16, tag="xr")
                    for j in range(C):
                        nc.gpsimd.indirect_dma_start(
                            out=xr[:, j * fdim:(j + 1) * fdim], out_offset=None,
                            in_=tab,
                            in_offset=bass.IndirectOffsetOnAxis(
                                ap=idx_all[:, b * C + j:b * C + j + 1], axis=0),
                        )
                    smat = build_smat(b)

                    aggT = [ppool.tile([P, P], f32, tag=f"agg{k}", name=f"aggps{k}")
                            for k in range(nf)]
                    for j in range(C):
                        for k in range(nf):
                            nc.tensor.matmul(
                                aggT[k][:],
                                lhsT=xr[:, j * fdim + k * P: j * fdim + (k + 1) * P],
                                rhs=smat[:, j * P:(j + 1) * P],
                                start=(j == 0), stop=(j == C - 1))

                    aggs = [wpool.tile([P, P], bf16, tag=f"aggs{k}", name=f"aggs{k}")
                            for k in range(nf)]
                    for k in range(nf):
                        nc.scalar.copy(aggs[k][:], aggT[k][:])

                    hrow = hpool.tile([P, HID], bf16, tag="hrow")
                    for h in range(2):
                        hT_ps = ppool.tile([P, P], f32, tag="ht")
                        for k in range(nf):
                            nc.tensor.matmul(
                                hT_ps[:], lhsT=wtiles[k][:, h * P:(h + 1) * P],
                                rhs=aggs[k][:], start=(k == 0), stop=(k == nf - 1))
                        hTs = wpool.tile([P, P], f32, tag=f"hTs{h}")
                        nc.scalar.activation(hTs[:], hT_ps[:], AF.Relu,
                                             bias=bias_t[:, bc0 + h:bc0 + h + 1])
                        if li == 0:
                            nc.vector.tensor_scalar(
                                out=resid[b][h][:], in0=hTs[:],
                                scalar1=tsh_t[:, bc0 + h:bc0 + h + 1], scalar2=None,
                                op0=OP.add)
                        else:
                            u = wpool.tile([P, P], bf16, tag=f"u{h}")
                            nc.vector.tensor_scalar(
                                out=u[:], in0=hTs[:],
                                scalar1=tsh_t[:, bc0 + h:bc0 + h + 1], scalar2=None,
                                op0=OP.add)
                            nc.vector.tensor_tensor(
                                out=resid[b][h][:], in0=resid[b][h][:], in1=u[:],
                                op=OP.add)
                        tp_ps = ppool.tile([P, P], bf16, tag="ht")
                        nc.tensor.transpose(tp_ps[:], resid[b][h][:], ident[:])
                        nc.scalar.copy(hrow[:, h * P:(h + 1) * P], tp_ps[:])

                    if hloc is not None:
                        nc.sync.dma_start(
                            out=hloc[b // AGRP][(b % AGRP) * P:(b % AGRP + 1) * P, :],
                            in_=hrow[:])
                    else:
                        # L3: pool inline. mblk[t, g] = (batch[t]==g)
                        mblk = spool.tile([P, N_GRAPHS], bf16, tag="mblk")
                        nc.vector.tensor_tensor(
                            out=mblk[:],
                            in0=bcol_t[:, b:b + 1].broadcast_to([P, N_GRAPHS]),
                            in1=iota5_f[:], op=OP.is_equal)
                        for h in range(2):
                            nc.tensor.matmul(
                                pooled_ps[h][:], lhsT=hrow[:, h * P:(h + 1) * P],
                                rhs=mblk[:], start=(b == 0), stop=(b == NBLK - 1))

            def allgather_tail(hloc, xnext):
                for g in range(NGRP):
                    ag_one(hloc, xnext, g)

            tab1 = x_in[:, :]
            layer(0, tab1, IN_DIM, idx1_t, [w1_t], 0, hloc1, xnext1)
            allgather_tail(hloc1, xnext1)
            tab2 = xnext1[:, :, :, :].rearrange("g c r f -> (g c r) f")
            layer(1, tab2, HID, idx2_t, w2_t, 2, hloc2, xnext2)
            allgather_tail(hloc2, xnext2)
            tab3 = xnext2[:, :, :, :].rearrange("g c r f -> (g c r) f")
            layer(2, tab3, HID, idx2_t, w3_t, 4, None)

            # pooled partial sums -> DRAM -> AllReduce
            for h in range(2):
                ps = wpool.tile([P, N_GRAPHS], f32, tag=f"poolsb{h}")
                nc.vector.tensor_copy(ps[:], pooled_ps[h][:])
                nc.sync.dma_start(out=prdram[h * P:(h + 1) * P, :], in_=ps[:])
            nc.gpsimd.collective_compute(
                "AllReduce", bass.mybir.AluOpType.add,
                replica_groups=[list(range(NCORES))],
                ins=[prdram[:, :]], outs=[ardram[:, :]])

            # head: h1T[o,g] = relu(lw1.T @ (pooledT*icnt) + lb1); out = lw2.T @ h1T + lb2
            lw1_t = [cpool.tile([P, HID], f32, tag=f"lw1_{k}", name=f"lw1_{k}") for k in range(2)]
            lw2_t = cpool.tile([P, 2], f32, tag="lw2")
            lb1_t = cpool.tile([P, 2], f32, tag="lb1")
            lb2_t = cpool.tile([1, 1], f32, tag="lb2")
            for k in range(2):
                nc.sync.dma_start(out=lw1_t[k][:], in_=lw1[k * P:(k + 1) * P, :])
            nc.sync.dma_start(out=lw2_t[:], in_=lw2[:, :])
            nc.sync.dma_start(out=lb1_t[:], in_=lb1c[:, :])
            nc.sync.dma_start(out=lb2_t[:], in_=lb2c[:, :])

            par = []
            for k in range(2):
                pk = wpool.tile([P, N_GRAPHS], f32, tag=f"par{k}")
                nc.sync.dma_start(out=pk[:], in_=ardram[k * P:(k + 1) * P, :])
                pks = wpool.tile([P, N_GRAPHS], f32, tag=f"pars{k}")
                nc.vector.tensor_tensor(out=pks[:], in0=pk[:], in1=icnt_t[:], op=OP.mult)
                par.append(pks)
            h1s = []
            for h in range(2):
                h1_ps = ppool.tile([P, N_GRAPHS], f32, tag="agg0")
                for k in range(2):
                    nc.tensor.matmul(h1_ps[:], lhsT=lw1_t[k][:, h * P:(h + 1) * P],
                                     rhs=par[k][:], start=(k == 0), stop=(k == 1))
                h1sb = wpool.tile([P, N_GRAPHS], f32, tag=f"h1s{h}")
                nc.scalar.activation(h1sb[:], h1_ps[:], AF.Relu,
                                     bias=lb1_t[:, h:h + 1])
                h1s.append(h1sb)
            out_ps = ppool.tile([1, N_GRAPHS], f32, tag="agg1")
            for h in range(2):
                nc.tensor.matmul(out_ps[:], lhsT=lw2_t[:, h:h + 1],
                                 rhs=h1s[h][:], start=(h == 0), stop=(h == 1))
            out_sb = wpool.tile([1, N_GRAPHS], f32, tag="outs")
            nc.vector.tensor_scalar(out=out_sb[:], in0=out_ps[:],
                                    scalar1=lb2_t[0:1, 0:1], scalar2=None, op0=OP.add)
            nc.sync.dma_start(out=out[:, :], in_=out_sb[:])

    nc.compile()
    return nc


def _preprocess(edge_index, batch):
    """Degree-balanced node->block assignment + per-core edge lists grouped by
    target block, padded to uniform chunks."""
    src = np.asarray(edge_index[0], dtype=np.int64)
    tgt = np.asarray(edge_index[1], dtype=np.int64)
    batch = np.asarray(batch, dtype=np.int64)

    indeg = np.bincount(tgt, minlength=N_NODES).astype(np.int64)
    deg = indeg.astype(np.float64) + 1.0
    dinv = 1.0 / np.sqrt(deg)

    # balanced snake assignment of 50176 slots (incl 176 weight-0 virtual)
    slots = XROWS
    wts = np.concatenate([indeg + 1, np.zeros(slots - N_NODES, np.int64)])
    order = np.argsort(-wts, kind="stable")
    assign_block = np.empty(slots, np.int64)
    fwd = np.arange(NTOT)
    for r in range(P):
        seg = order[r * NTOT:(r + 1) * NTOT]
        assign_block[seg] = fwd if r % 2 == 0 else fwd[::-1]
    perm = np.argsort(assign_block, kind="stable")
    newpos = np.empty(slots, np.int64)
    newpos[perm] = np.arange(slots)

    allsrc = np.concatenate([src, np.arange(N_NODES, dtype=np.int64)])
    alltgt = np.concatenate([tgt, np.arange(N_NODES, dtype=np.int64)])
    allw = (dinv[allsrc] * dinv[alltgt]).astype(np.float32)

    tgt_np = newpos[alltgt]
    blkkey = tgt_np // P
    order_e = np.argsort(blkkey, kind="stable")
    allsrc, alltgt, allw = allsrc[order_e], alltgt[order_e], allw[order_e]
    tgt_np = tgt_np[order_e]

    counts = np.bincount(blkkey, minlength=NTOT)
    chunks = int(math.ceil(counts.max() / P))
    C = chunks

    GR = AGRP * P
    blk_start = np.zeros(NTOT + 1, dtype=np.int64)
    np.cumsum(counts, out=blk_start[1:])

    import ml_dtypes
    per_core = []
    for c in range(NCORES):
        idx1 = np.zeros((NBLK, P, C), dtype=np.int32)
        idx2 = np.zeros((NBLK, P, C), dtype=np.int32)
        meta = np.zeros((NBLK, P, 2 * C), dtype=np.float32)
        for b in range(NBLK):
            g = c * NBLK + b
            lo, hi = blk_start[g], blk_start[g + 1]
            n = hi - lo
            s1 = allsrc[lo:hi].astype(np.int32)
            np_src = newpos[allsrc[lo:hi]]
            core_s = np_src // PADN
            r_s = np_src % PADN
            s2 = ((r_s // GR) * (NCORES * GR) + core_s * GR + (r_s % GR)
                  ).astype(np.int32)
            tl = (tgt_np[lo:hi] - g * P).astype(np.float32)
            ww = allw[lo:hi]
            npad = C * P - n
            if npad:
                s1 = np.pad(s1, (0, npad))
                s2 = np.pad(s2, (0, npad))
                tl = np.pad(tl, (0, npad))
                ww = np.pad(ww, (0, npad))
            idx1[b] = s1.reshape(C, P).T
            idx2[b] = s2.reshape(C, P).T
            meta[b, :, :C] = tl.reshape(C, P).T
            meta[b, :, C:] = ww.reshape(C, P).T
        # batch column for pooling (pad/virtual rows -> -1)
        core_slots = perm[c * PADN:(c + 1) * PADN]   # orig ids in new order
        bvals = np.where(core_slots < N_NODES,
                         batch[np.minimum(core_slots, N_NODES - 1)], -1.0)
        bcol = bvals.reshape(NBLK, P).T.astype(np.float32)  # [P, NBLK]
        per_core.append(dict(
            idx1=idx1.transpose(1, 0, 2).reshape(P, NBLK * C).copy(),
            idx2=idx2.transpose(1, 0, 2).reshape(P, NBLK * C).copy(),
            meta=meta.transpose(1, 0, 2).reshape(P, NBLK * 2 * C)
                .astype(ml_dtypes.bfloat16),
            bcol=bcol.copy(),
        ))
    return per_core, chunks


def kernel(**inputs):
    import ml_dtypes
    from concourse.bass_utils import run_bass_kernel_spmd

    x = np.asarray(inputs["x"], dtype=np.float32)
    edge_index = np.asarray(inputs["edge_index"])
    batch = np.asarray(inputs["batch"])

    per_core, chunks = _preprocess(edge_index, batch)

    def g(k):
        return np.asarray(inputs[k], dtype=np.float32)

    params = {}
    Ws = [g("W1"), g("W2"), g("W3")]
    bs = [g("b1"), g("b2"), g("b3")]
    bias = np.zeros((P, 6), np.float32)
    tshv = np.zeros((P, 6), np.float32)
    wp = []
    for i in range(3):
        gam, be, m, v = g(f"g{i+1}"), g(f"be{i+1}"), g(f"m{i+1}"), g(f"v{i+1}")
        s = gam / np.sqrt(v + BN_EPS)
        assert (s > 0).all(), "BN scale must be positive for relu folding"
        wp.append((Ws[i] * s[None, :]).astype(ml_dtypes.bfloat16))
        bp = (bs[i] * s).astype(np.float32)
        tv = (be - m * s).astype(np.float32)
        bias[:, 2 * i] = bp[:P]
        bias[:, 2 * i + 1] = bp[P:]
        tshv[:, 2 * i] = tv[:P]
        tshv[:, 2 * i + 1] = tv[P:]
    params["w1p"], params["w2p"], params["w3p"] = wp
    params["bias"] = bias
    params["tsh"] = tshv
    params["lw1"] = g("lw1")
    lb1 = g("lb1")
    lb1c = np.zeros((P, 2), np.float32)
    lb1c[:, 0] = lb1[:P]
    lb1c[:, 1] = lb1[P:]
    params["lb1c"] = lb1c
    lw2v = g("lw2").reshape(HID)
    params["lw2"] = np.stack([lw2v[:P], lw2v[P:]], axis=1).copy()
    params["lb2c"] = g("lb2").reshape(1, 1).astype(np.float32)
    cnt = np.bincount(np.asarray(batch, dtype=np.int64), minlength=N_GRAPHS)
    icnt = (1.0 / np.maximum(cnt, 1)).astype(np.float32)
    params["icnt"] = np.tile(icnt[None, :], (P, 1))
    x_bf = x.astype(ml_dtypes.bfloat16)

    nc = _build_program(chunks)

    in_maps = []
    for c in range(NCORES):
        m = dict(params)
        m["x"] = x_bf
        m.update(per_core[c])
        in_maps.append(m)

    res = run_bass_kernel_spmd(nc, in_maps, list(range(NCORES)),
                               trace=bool(os.environ.get("GNN_TRACE")))
    if os.environ.get("GNN_TRACE"):
        print("HW exec time:", res.exec_time_ns, "ns")
    global _last_results
    _last_results = res.results
    o = res.results[0]["out"]
    return np.asarray(o, dtype=np.float32).reshape(N_GRAPHS, OUT_DIM)


# revision 11
# speedup vs baseline: 1.0929x; 1.0118x over previous
"""GCN (3x GCNConv + BN + residual, mean-pool, MLP head) on 8 trn2 NeuronCores.

Sharding: nodes are assigned to 392 blocks of 128 via degree-balanced snake
packing (equalizes per-block incident-edge counts, minimizing gather-chunk
padding); 49 blocks per core. Each core owns the edges whose TARGET lands in
its blocks (plus self-loops). GCN normalization is linear, so each layer
aggregates raw input features over incident edges (one indirect-DMA gather of
128 source rows + one PE matmul with a selection matrix per 128-edge chunk),
then applies the folded linear+BN epilogue. Activation tables are bf16;
AllGathers between layers are chunked (7 groups of 7 blocks) so they overlap
the producing layer's compute. Per-graph pooled sums are AllReduced; the tiny
MLP head runs redundantly on every core.

Device kernel per (layer, target-block of 128 nodes):
  for each 128-edge chunk: indirect-DMA gather of source rows (bf16);
  one fused DVE pass builds all selection matrices S[e,t] = w'[e]*(tl[e]==t);
  PE: aggT[f,t] += xr[:,f-chunk].T @ S_j (PSUM, bf16 inputs);
  hT[o,t] = sum_f W'[f,o].T @ agg[f,t]; ACT relu + folded bias; DVE +tsh
  (+residual); PE transpose back to [t,o] bf16 rows for the next layer's
  gather table / pooling.
"""
import math
import os
import sys

import numpy as np

sys.path.insert(0, "/opt/trn_rl_repo")

N_NODES = 50000
N_EDGES = 800000
IN_DIM = 128
HID = 256
OUT_DIM = 1
N_GRAPHS = 512
BN_EPS = 1e-5
NCORES = 8
P = 128
NBLK = 49                            # blocks per core
NTOT = NBLK * NCORES                 # 392 blocks globally
PADN = NBLK * P                      # 6272 rows per core (incl pad slots)
XROWS = PADN * NCORES                # 50176 rows in allgathered tables
AGRP = 7                             # blocks per chunked-AllGather group
NGRP = NBLK // AGRP                  # 7 groups


def _build_program(chunks):
    from concourse import bass, bacc, mybir, tile
    from concourse.masks import make_identity

    f32 = mybir.dt.float32
    bf16 = mybir.dt.bfloat16
    i32 = mybir.dt.int32
    AF = mybir.ActivationFunctionType
    OP = mybir.AluOpType

    C = chunks
    GRows = AGRP * P                 # rows per AllGather group (896)

    nc = bacc.Bacc("TRN2", target_bir_lowering=False, debug=False,
                   num_devices=NCORES)

    x_in = nc.declare_dram_parameter("x", [N_NODES, IN_DIM], bf16, isOutput=False)
    idx1 = nc.declare_dram_parameter("idx1", [P, NBLK * C], i32, isOutput=False)
    idx2 = nc.declare_dram_parameter("idx2", [P, NBLK * C], i32, isOutput=False)
    meta = nc.declare_dram_parameter("meta", [P, NBLK * 2 * C], bf16, isOutput=False)
    bcol = nc.declare_dram_parameter("bcol", [P, NBLK], f32, isOutput=False)
    w1p = nc.declare_dram_parameter("w1p", [IN_DIM, HID], bf16, isOutput=False)
    w2p = nc.declare_dram_parameter("w2p", [HID, HID], bf16, isOutput=False)
    w3p = nc.declare_dram_parameter("w3p", [HID, HID], bf16, isOutput=False)
    bias = nc.declare_dram_parameter("bias", [P, 6], f32, isOutput=False)
    tsh = nc.declare_dram_parameter("tsh", [P, 6], f32, isOutput=False)
    lw1 = nc.declare_dram_parameter("lw1", [HID, HID], f32, isOutput=False)
    lb1c = nc.declare_dram_parameter("lb1c", [P, 2], f32, isOutput=False)
    lw2 = nc.declare_dram_parameter("lw2", [P, 2], f32, isOutput=False)
    lb2c = nc.declare_dram_parameter("lb2c", [1, 1], f32, isOutput=False)
    icnt = nc.declare_dram_parameter("icnt", [P, N_GRAPHS], f32, isOutput=False)
    out = nc.declare_dram_parameter("out", [1, N_GRAPHS], f32, isOutput=True)

    with tile.TileContext(nc) as tc:
        with tc.tile_pool(name="const", bufs=1) as cpool, \
             tc.tile_pool(name="rows", bufs=6) as rpool, \
             tc.tile_pool(name="smat", bufs=2) as spool, \
             tc.tile_pool(name="work", bufs=2) as wpool, \
             tc.tile_pool(name="resid", bufs=1) as residp, \
             tc.tile_pool(name="hrow", bufs=3) as hpool, \
             tc.tile_pool(name="psum", bufs=2, space="PSUM") as ppool, \
             tc.tile_pool(name="psump", bufs=1, space="PSUM") as ppoolp, \
             tc.tile_pool(name="dram", bufs=1, space="DRAM") as dpool:

            iota_i = cpool.tile([P, P], i32, tag="ioi")
            nc.gpsimd.iota(iota_i[:], pattern=[[1, P]], base=0, channel_multiplier=0)
            iota_b = cpool.tile([P, P], bf16, tag="iob")
            nc.vector.tensor_copy(iota_b[:], iota_i[:])
            iota5_i = cpool.tile([P, N_GRAPHS], i32, tag="io5i")
            nc.gpsimd.iota(iota5_i[:], pattern=[[1, N_GRAPHS]], base=0, channel_multiplier=0)
            iota5_f = cpool.tile([P, N_GRAPHS], f32, tag="io5f")
            nc.vector.tensor_copy(iota5_f[:], iota5_i[:])
            ident = cpool.tile([P, P], bf16, tag="ident")
            make_identity(nc, ident[:])

            bias_t = cpool.tile([P, 6], f32, tag="bias")
            nc.sync.dma_start(out=bias_t[:], in_=bias[:, :])
            tsh_t = cpool.tile([P, 6], f32, tag="tsh")
            nc.sync.dma_start(out=tsh_t[:], in_=tsh[:, :])

            w1_t = cpool.tile([IN_DIM, HID], bf16, tag="w1")
            nc.sync.dma_start(out=w1_t[:], in_=w1p[:, :])
            w2_t = [cpool.tile([P, HID], bf16, tag=f"w2_{k}", name=f"w2_{k}") for k in range(2)]
            w3_t = [cpool.tile([P, HID], bf16, tag=f"w3_{k}", name=f"w3_{k}") for k in range(2)]
            for k in range(2):
                nc.sync.dma_start(out=w2_t[k][:], in_=w2p[k * P:(k + 1) * P, :])
                nc.sync.dma_start(out=w3_t[k][:], in_=w3p[k * P:(k + 1) * P, :])

            # all per-block metadata loaded upfront (small)
            idx1_t = cpool.tile([P, NBLK * C], i32, tag="idx1")
            nc.sync.dma_start(out=idx1_t[:], in_=idx1[:, :])
            idx2_t = cpool.tile([P, NBLK * C], i32, tag="idx2")
            nc.sync.dma_start(out=idx2_t[:], in_=idx2[:, :])
            meta_t = cpool.tile([P, NBLK * 2 * C], bf16, tag="meta")
            nc.sync.dma_start(out=meta_t[:], in_=meta[:, :])
            bcol_t = cpool.tile([P, NBLK], f32, tag="bcol")
            nc.sync.dma_start(out=bcol_t[:], in_=bcol[:, :])
            icnt_t = cpool.tile([P, N_GRAPHS], f32, tag="icnt")
            nc.sync.dma_start(out=icnt_t[:], in_=icnt[:, :])

            hloc1 = [dpool.tile([GRows, HID], bf16, tag=f"hloc1_{g}",
                                 name=f"hloc1_{g}") for g in range(NGRP)]
            hloc2 = [dpool.tile([GRows, HID], bf16, tag=f"hloc2_{g}",
                                 name=f"hloc2_{g}") for g in range(NGRP)]
            xnext1 = dpool.tile([NGRP, NCORES, GRows, HID], bf16, tag="xn1")
            xnext2 = dpool.tile([NGRP, NCORES, GRows, HID], bf16, tag="xn2")
            prdram = dpool.tile([HID, N_GRAPHS], f32, tag="prd")
            ardram = dpool.tile([HID, N_GRAPHS], f32, tag="ard")

            resid = [[residp.tile([P, P], bf16, tag=f"r{b}h{h}", name=f"r{b}h{h}")
                      for h in range(2)] for b in range(NBLK)]

            pooled_ps = [ppoolp.tile([P, N_GRAPHS], f32, tag=f"pool{h}", name=f"pool{h}")
                         for h in range(2)]

            def build_smat(b):
                """One fused DVE pass per block: S[e, j*128+t] = w'[j] * (tl[j]==t)."""
                s01 = spool.tile([P, C * P], bf16, tag="s01")
                smat = spool.tile([P, C * P], bf16, tag="smat")
                tl_ap = meta_t[:, b * 2 * C: b * 2 * C + C].unsqueeze(2) \
                    .broadcast_to([P, C, P])
                w_ap = meta_t[:, b * 2 * C + C: (b + 1) * 2 * C].unsqueeze(2) \
                    .broadcast_to([P, C, P])
                io_ap = iota_b[:, :].unsqueeze(1).broadcast_to([P, C, P])
                s01_3d = s01[:].rearrange("p (c t) -> p c t", c=C)
                smat_3d = smat[:].rearrange("p (c t) -> p c t", c=C)
                nc.vector.tensor_tensor(out=s01_3d, in0=tl_ap, in1=io_ap,
                                        op=mybir.AluOpType.is_equal)
                nc.vector.tensor_tensor(out=smat_3d, in0=s01_3d, in1=w_ap,
                                        op=mybir.AluOpType.mult)
                return smat

            def ag_one(hloc, xnext, g):
                nc.gpsimd.collective_compute(
                    "AllGather", bass.mybir.AluOpType.bypass,
                    replica_groups=[list(range(NCORES))],
                    ins=[hloc[g][:, :]], outs=[xnext[g]])

            def layer(li, tab, fdim, idx_all, wtiles, bc0, hloc, xnext=None):
                nf = fdim // P
                for b in range(NBLK):
                    xr = rpool.tile([P, C * fdim], bf16, tag="xr")
                    for j in range(C):
                        nc.gpsimd.indirect_dma_start(
                            out=xr[:, j * fdim:(j + 1) * fdim], out_offset=None,
                            in_=tab,
                            in_offset=bass.IndirectOffsetOnAxis(
                                ap=idx_all[:, b * C + j:b * C + j + 1], axis=0),
                        )
                    smat = build_smat(b)

                    aggT = [ppool.tile([P, P], f32, tag=f"agg{k}", name=f"aggps{k}")
                            for k in range(nf)]
                    for j in range(C):
                        for k in range(nf):
                            nc.tensor.matmul(
                                aggT[k][:],
                                lhsT=xr[:, j * fdim + k * P: j * fdim + (k + 1) * P],
                                rhs=smat[:, j * P:(j + 1) * P],
                                start=(j == 0), stop=(j == C - 1))

                    aggs = [wpool.tile([P, P], bf16, tag=f"aggs{k}", name=f"aggs{k}")
                            for k in range(nf)]
                    for k in range(nf):
                        nc.scalar.copy(aggs[k][:], aggT[k][:])

                    hrow = hpool.tile([P, HID], bf16, tag="hrow")
                    for h in range(2):
                        hT_ps = ppool.tile([P, P], f32, tag="ht")
                        for k in range(nf):
                            nc.tensor.matmul(
                                hT_ps[:], lhsT=wtiles[k][:, h * P:(h + 1) * P],
                                rhs=aggs[k][:], start=(k == 0), stop=(k == nf - 1))
                        hTs = wpool.tile([P, P], f32, tag=f"hTs{h}")
                        nc.scalar.activation(hTs[:], hT_ps[:], AF.Relu,
                                             bias=bias_t[:, bc0 + h:bc0 + h + 1])
                        if li == 0:
                            nc.vector.tensor_scalar(
                                out=resid[b][h][:], in0=hTs[:],
                                scalar1=tsh_t[:, bc0 + h:bc0 + h + 1], scalar2=None,
                                op0=OP.add)
                        else:
                            u = wpool.tile([P, P], bf16, tag=f"u{h}")
                            nc.vector.tensor_scalar(
                                out=u[:], in0=hTs[:],
                                scalar1=tsh_t[:, bc0 + h:bc0 + h + 1], scalar2=None,
                                op0=OP.add)
                            nc.vector.tensor_tensor(
                                out=resid[b][h][:], in0=resid[b][h][:], in1=u[:],
                                op=OP.add)
                        tp_ps = ppool.tile([P, P], bf16, tag="ht")
                        nc.tensor.transpose(tp_ps[:], resid[b][h][:], ident[:])
                        nc.scalar.copy(hrow[:, h * P:(h + 1) * P], tp_ps[:])

                    if hloc is not None:
                        nc.sync.dma_start(
                            out=hloc[b // AGRP][(b % AGRP) * P:(b % AGRP + 1) * P, :],
                            in_=hrow[:])
                    else:
                        # L3: pool inline. mblk[t, g] = (batch[t]==g)
                        mblk = spool.tile([P, N_GRAPHS], bf16, tag="mblk")
                        nc.vector.tensor_tensor(
                            out=mblk[:],
                            in0=bcol_t[:, b:b + 1].broadcast_to([P, N_GRAPHS]),
                            in1=iota5_f[:], op=OP.is_equal)
                        for h in range(2):
                            nc.tensor.matmul(
                                pooled_ps[h][:], lhsT=hrow[:, h * P:(h + 1) * P],
                                rhs=mblk[:], start=(b == 0), stop=(b == NBLK - 1))

            def allgather_tail(hloc, xnext):
                for g in range(NGRP):
                    ag_one(hloc, xnext, g)

            tab1 = x_in[:, :]
            layer(0, tab1, IN_DIM, idx1_t, [w1_t], 0, hloc1, xnext1)
            allgather_tail(hloc1, xnext1)
            tab2 = xnext1[:, :, :, :].rearrange("g c r f -> (g c r) f")
            layer(1, tab2, HID, idx2_t, w2_t, 2, hloc2, xnext2)
            allgather_tail(hloc2, xnext2)
            tab3 = xnext2[:, :, :, :].rearrange("g c r f -> (g c r) f")
            layer(2, tab3, HID, idx2_t, w3_t, 4, None)

            # pooled partial sums -> DRAM -> AllReduce
            for h in range(2):
                ps = wpool.tile([P, N_GRAPHS], f32, tag=f"poolsb{h}")
                nc.vector.tensor_copy(ps[:], pooled_ps[h][:])
                nc.sync.dma_start(out=prdram[h * P:(h + 1) * P, :], in_=ps[:])
            nc.gpsimd.collective_compute(
                "AllReduce", bass.mybir.AluOpType.add,
                replica_groups=[list(range(NCORES))],
                ins=[prdram[:, :]], outs=[ardram[:, :]])

            # head: h1T[o,g] = relu(lw1.T @ (pooledT*icnt) + lb1); out = lw2.T @ h1T + lb2
            lw1_t = [cpool.tile([P, HID], f32, tag=f"lw1_{k}", name=f"lw1_{k}") for k in range(2)]
            lw2_t = cpool.tile([P, 2], f32, tag="lw2")
            lb1_t = cpool.tile([P, 2], f32, tag="lb1")
            lb2_t = cpool.tile([1, 1], f32, tag="lb2")
            for k in range(2):
                nc.sync.dma_start(out=lw1_t[k][:], in_=lw1[k * P:(k + 1) * P, :])
            nc.sync.dma_start(out=lw2_t[:], in_=lw2[:, :])
            nc.sync.dma_start(out=lb1_t[:], in_=lb1c[:, :])
            nc.sync.dma_start(out=lb2_t[:], in_=lb2c[:, :])

            par = []
            for k in range(2):
                pk = wpool.tile([P, N_GRAPHS], f32, tag=f"par{k}")
                nc.sync.dma_start(out=pk[:], in_=ardram[k * P:(k + 1) * P, :])
                pks = wpool.tile([P, N_GRAPHS], f32, tag=f"pars{k}")
                nc.vector.tensor_tensor(out=pks[:], in0=pk[:], in1=icnt_t[:], op=OP.mult)
                par.append(pks)
            h1s = []
            for h in range(2):
                h1_ps = ppool.tile([P, N_GRAPHS], f32, tag="agg0")
                for k in range(2):
                    nc.tensor.matmul(h1_ps[:], lhsT=lw1_t[k][:, h * P:(h + 1) * P],
                                     rhs=par[k][:], start=(k == 0), stop=(k == 1))
                h1sb = wpool.tile([P, N_GRAPHS], f32, tag=f"h1s{h}")
                nc.scalar.activation(h1sb[:], h1_ps[:], AF.Relu,
                                     bias=lb1_t[:, h:h + 1])
                h1s.append(h1sb)
            out_ps = ppool.tile([1, N_GRAPHS], f32, tag="agg1")
            for h in range(2):
                nc.tensor.matmul(out_ps[:], lhsT=lw2_t[:, h:h + 1],
                                 rhs=h1s[h][:], start=(h == 0), stop=(h == 1))
            out_sb = wpool.tile([1, N_GRAPHS], f32, tag="outs")
            nc.vector.tensor_scalar(out=out_sb[:], in0=out_ps[:],
                                    scalar1=lb2_t[0:1, 0:1], scalar2=None, op0=OP.add)
            nc.sync.dma_start(out=out[:, :], in_=out_sb[:])

    nc.compile()
    return nc


def _preprocess(edge_index, batch):
    """Degree-balanced node->block assignment + per-core edge lists grouped by
    target block, padded to uniform chunks."""
    src = np.asarray(edge_index[0], dtype=np.int64)
    tgt = np.asarray(edge_index[1], dtype=np.int64)
    batch = np.asarray(batch, dtype=np.int64)

    indeg = np.bincount(tgt, minlength=N_NODES).astype(np.int64)
    deg = indeg.astype(np.float64) + 1.0
    dinv = 1.0 / np.sqrt(deg)

    # balanced snake assignment of 50176 slots (incl 176 weight-0 virtual)
    slots = XROWS
    wts = np.concatenate([indeg + 1, np.zeros(slots - N_NODES, np.int64)])
    order = np.argsort(-wts, kind="stable")
    assign_block = np.empty(slots, np.int64)
    fwd = np.arange(NTOT)
    for r in range(P):
        seg = order[r * NTOT:(r + 1) * NTOT]
        assign_block[seg] = fwd if r % 2 == 0 else fwd[::-1]
    perm = np.argsort(assign_block, kind="stable")
    newpos = np.empty(slots, np.int64)
    newpos[perm] = np.arange(slots)

    allsrc = np.concatenate([src, np.arange(N_NODES, dtype=np.int64)])
    alltgt = np.concatenate([tgt, np.arange(N_NODES, dtype=np.int64)])
    allw = (dinv[allsrc] * dinv[alltgt]).astype(np.float32)

    tgt_np = newpos[alltgt]
    blkkey = tgt_np // P
    order_e = np.argsort(blkkey, kind="stable")
    allsrc, alltgt, allw = allsrc[order_e], alltgt[order_e], allw[order_e]
    tgt_np = tgt_np[order_e]

    counts = np.bincount(blkkey, minlength=NTOT)
    chunks = int(math.ceil(counts.max() / P))
    C = chunks

    GR = AGRP * P
    blk_start = np.zeros(NTOT + 1, dtype=np.int64)
    np.cumsum(counts, out=blk_start[1:])

    import ml_dtypes
    per_core = []
    for c in range(NCORES):
        idx1 = np.zeros((NBLK, P, C), dtype=np.int32)
        idx2 = np.zeros((NBLK, P, C), dtype=np.int32)
        meta = np.zeros((NBLK, P, 2 * C), dtype=np.float32)
        for b in range(NBLK):
            g = c * NBLK + b
            lo, hi = blk_start[g], blk_start[g + 1]
            n = hi - lo
            s1 = allsrc[lo:hi].astype(np.int32)
            np_src = newpos[allsrc[lo:hi]]
            core_s = np_src // PADN
            r_s = np_src % PADN
            s2 = ((r_s // GR) * (NCORES * GR) + core_s * GR + (r_s % GR)
                  ).astype(np.int32)
            tl = (tgt_np[lo:hi] - g * P).astype(np.float32)
            ww = allw[lo:hi]
            npad = C * P - n
            if npad:
                s1 = np.pad(s1, (0, npad))
                s2 = np.pad(s2, (0, npad))
                tl = np.pad(tl, (0, npad))
                ww = np.pad(ww, (0, npad))
            idx1[b] = s1.reshape(C, P).T
            idx2[b] = s2.reshape(C, P).T
            meta[b, :, :C] = tl.reshape(C, P).T
            meta[b, :, C:] = ww.reshape(C, P).T
        # batch column for pooling (pad/virtual rows -> -1)
        core_slots = perm[c * PADN:(c + 1) * PADN]   # orig ids in new order
        bvals = np.where(core_slots < N_NODES,
                         batch[np.minimum(core_slots, N_NODES - 1)], -1.0)
        bcol = bvals.reshape(NBLK, P).T.astype(np.float32)  # [P, NBLK]
        per_core.append(dict(
            idx1=idx1.transpose(1, 0, 2).reshape(P, NBLK * C).copy(),
            idx2=idx2.transpose(1, 0, 2).reshape(P, NBLK * C).copy(),
            meta=meta.transpose(1, 0, 2).reshape(P, NBLK * 2 * C)
                .astype(ml_dtypes.bfloat16),
            bcol=bcol.copy(),
        ))
    return per_core, chunks


def kernel(**inputs):
    import ml_dtypes
    from concourse.bass_utils import run_bass_kernel_spmd

    x = np.asarray(inputs["x"], dtype=np.float32)
    edge_index = np.asarray(inputs["edge_index"])
    batch = np.asarray(inputs["batch"])

    per_core, chunks = _preprocess(edge_index, batch)

    def g(k):
        return np.asarray(inputs[k], dtype=np.float32)

    params = {}
    Ws = [g("W1"), g("W2"), g("W3")]
    bs = [g("b1"), g("b2"), g("b3")]
    bias = np.zeros((P, 6), np.float32)
    tshv = np.zeros((P, 6), np.float32)
    wp = []
    for i in range(3):
        gam, be, m, v = g(f"g{i+1}"), g(f"be{i+1}"), g(f"m{i+1}"), g(f"v{i+1}")
        s = gam / np.sqrt(v + BN_EPS)
        assert (s > 0).all(), "BN scale must be positive for relu folding"
        wp.append((Ws[i] * s[None, :]).astype(ml_dtypes.bfloat16))
        bp = (bs[i] * s).astype(np.float32)
        tv = (be - m * s).astype(np.float32)
        bias[:, 2 * i] = bp[:P]
        bias[:, 2 * i + 1] = bp[P:]
        tshv[:, 2 * i] = tv[:P]
        tshv[:, 2 * i + 1] = tv[P:]
    params["w1p"], params["w2p"], params["w3p"] = wp
    params["bias"] = bias
    params["tsh"] = tshv
    params["lw1"] = g("lw1")
    lb1 = g("lb1")
    lb1c = np.zeros((P, 2), np.float32)
    lb1c[:, 0] = lb1[:P]
    lb1c[:, 1] = lb1[P:]
    params["lb1c"] = lb1c
    lw2v = g("lw2").reshape(HID)
    params["lw2"] = np.stack([lw2v[:P], lw2v[P:]], axis=1).copy()
    params["lb2c"] = g("lb2").reshape(1, 1).astype(np.float32)
    cnt = np.bincount(np.asarray(batch, dtype=np.int64), minlength=N_GRAPHS)
    icnt = (1.0 / np.maximum(cnt, 1)).astype(np.float32)
    params["icnt"] = np.tile(icnt[None, :], (P, 1))
    x_bf = x.astype(ml_dtypes.bfloat16)

    nc = _build_program(chunks)

    in_maps = []
    for c in range(NCORES):
        m = dict(params)
        m["x"] = x_bf
        m.update(per_core[c])
        in_maps.append(m)

    res = run_bass_kernel_spmd(nc, in_maps, list(range(NCORES)),
                               trace=bool(os.environ.get("GNN_TRACE")))
    if os.environ.get("GNN_TRACE"):
        print("HW exec time:", res.exec_time_ns, "ns")
    global _last_results
    _last_results = res.results
    o = res.results[0]["out"]
    return np.asarray(o, dtype=np.float32).reshape(N_GRAPHS, OUT_DIM)


# revision 12
# speedup vs baseline: 1.1144x; 1.0197x over previous
"""GCN (3x GCNConv + BN + residual, mean-pool, MLP head) on 8 trn2 NeuronCores.

Sharding: nodes are assigned to 392 blocks of 128 via degree-balanced snake
packing (equalizes per-block incident-edge counts, minimizing gather-chunk
padding); 49 blocks per core. Each core owns the edges whose TARGET lands in
its blocks (plus self-loops). GCN normalization is linear, so each layer
aggregates raw input features over incident edges (one indirect-DMA gather of
128 source rows + one PE matmul with a selection matrix per 128-edge chunk),
then applies the folded linear+BN epilogue. Activation tables are bf16;
AllGathers between layers are chunked (7 groups of 7 blocks) so they overlap
the producing layer's compute. Per-graph pooled sums are AllReduced; the tiny
MLP head runs redundantly on every core.

Device kernel per (layer, target-block of 128 nodes):
  for each 128-edge chunk: indirect-DMA gather of source rows (bf16);
  one fused DVE pass builds all selection matrices S[e,t] = w'[e]*(tl[e]==t);
  PE: aggT[f,t] += xr[:,f-chunk].T @ S_j (PSUM, bf16 inputs);
  hT[o,t] = sum_f W'[f,o].T @ agg[f,t]; ACT relu + folded bias; DVE +tsh
  (+residual); PE transpose back to [t,o] bf16 rows for the next layer's
  gather table / pooling.
"""
import math
import os
import sys

import numpy as np

sys.path.insert(0, "/opt/trn_rl_repo")

N_NODES = 50000
N_EDGES = 800000
IN_DIM = 128
HID = 256
OUT_DIM = 1
N_GRAPHS = 512
BN_EPS = 1e-5
NCORES = 8
P = 128
NBLK = 49                            # blocks per core
NTOT = NBLK * NCORES                 # 392 blocks globally
PADN = NBLK * P                      # 6272 rows per core (incl pad slots)
XROWS = PADN * NCORES                # 50176 rows in allgathered tables
AGRP = 7                             # blocks per chunked-AllGather group
NGRP = NBLK // AGRP                  # 7 groups


def _build_program(chunks):
    from concourse import bass, bacc, mybir, tile
    from concourse.masks import make_identity

    f32 = mybir.dt.float32
    bf16 = mybir.dt.bfloat16
    i32 = mybir.dt.int32
    AF = mybir.ActivationFunctionType
    OP = mybir.AluOpType

    C = chunks
    GRows = AGRP * P                 # rows per AllGather group (896)

    nc = bacc.Bacc("TRN2", target_bir_lowering=False, debug=False,
                   num_devices=NCORES)

    x_in = nc.declare_dram_parameter("x", [N_NODES, IN_DIM], bf16, isOutput=False)
    idx1 = nc.declare_dram_parameter("idx1", [P, NBLK * C], i32, isOutput=False)
    idx2 = nc.declare_dram_parameter("idx2", [P, NBLK * C], i32, isOutput=False)
    meta = nc.declare_dram_parameter("meta", [P, NBLK * 2 * C], bf16, isOutput=False)
    bcol = nc.declare_dram_parameter("bcol", [P, NBLK], f32, isOutput=False)
    w1p = nc.declare_dram_parameter("w1p", [IN_DIM, HID], bf16, isOutput=False)
    w2p = nc.declare_dram_parameter("w2p", [HID, HID], bf16, isOutput=False)
    w3p = nc.declare_dram_parameter("w3p", [HID, HID], bf16, isOutput=False)
    bias = nc.declare_dram_parameter("bias", [P, 6], f32, isOutput=False)
    tsh = nc.declare_dram_parameter("tsh", [P, 6], f32, isOutput=False)
    lw1 = nc.declare_dram_parameter("lw1", [HID, HID], f32, isOutput=False)
    lb1c = nc.declare_dram_parameter("lb1c", [P, 2], f32, isOutput=False)
    lw2 = nc.declare_dram_parameter("lw2", [P, 2], f32, isOutput=False)
    lb2c = nc.declare_dram_parameter("lb2c", [1, 1], f32, isOutput=False)
    icnt = nc.declare_dram_parameter("icnt", [P, N_GRAPHS], f32, isOutput=False)
    out = nc.declare_dram_parameter("out", [1, N_GRAPHS], f32, isOutput=True)

    with tile.TileContext(nc) as tc:
        with tc.tile_pool(name="const", bufs=1) as cpool, \
             tc.tile_pool(name="rows", bufs=6) as rpool, \
             tc.tile_pool(name="smat", bufs=2) as spool, \
             tc.tile_pool(name="work", bufs=2) as wpool, \
             tc.tile_pool(name="resid", bufs=1) as residp, \
             tc.tile_pool(name="hrow", bufs=3) as hpool, \
             tc.tile_pool(name="psum", bufs=2, space="PSUM") as ppool, \
             tc.tile_pool(name="psump", bufs=1, space="PSUM") as ppoolp, \
             tc.tile_pool(name="dram", bufs=1, space="DRAM") as dpool:

            iota_i = cpool.tile([P, P], i32, tag="ioi")
            nc.gpsimd.iota(iota_i[:], pattern=[[1, P]], base=0, channel_multiplier=0)
            iota_b = cpool.tile([P, P], bf16, tag="iob")
            nc.vector.tensor_copy(iota_b[:], iota_i[:])
            iota5_i = cpool.tile([P, N_GRAPHS], i32, tag="io5i")
            nc.gpsimd.iota(iota5_i[:], pattern=[[1, N_GRAPHS]], base=0, channel_multiplier=0)
            iota5_f = cpool.tile([P, N_GRAPHS], f32, tag="io5f")
            nc.vector.tensor_copy(iota5_f[:], iota5_i[:])
            ident = cpool.tile([P, P], bf16, tag="ident")
            make_identity(nc, ident[:])

            bias_t = cpool.tile([P, 6], f32, tag="bias")
            nc.sync.dma_start(out=bias_t[:], in_=bias[:, :])
            tsh_t = cpool.tile([P, 6], f32, tag="tsh")
            nc.sync.dma_start(out=tsh_t[:], in_=tsh[:, :])

            w1_t = cpool.tile([IN_DIM, HID], bf16, tag="w1")
            nc.sync.dma_start(out=w1_t[:], in_=w1p[:, :])
            w2_t = [cpool.tile([P, HID], bf16, tag=f"w2_{k}", name=f"w2_{k}") for k in range(2)]
            w3_t = [cpool.tile([P, HID], bf16, tag=f"w3_{k}", name=f"w3_{k}") for k in range(2)]
            for k in range(2):
                nc.sync.dma_start(out=w2_t[k][:], in_=w2p[k * P:(k + 1) * P, :])
                nc.sync.dma_start(out=w3_t[k][:], in_=w3p[k * P:(k + 1) * P, :])

            # all per-block metadata loaded upfront (small)
            idx1_t = cpool.tile([P, NBLK * C], i32, tag="idx1")
            nc.sync.dma_start(out=idx1_t[:], in_=idx1[:, :])
            idx2_t = cpool.tile([P, NBLK * C], i32, tag="idx2")
            nc.sync.dma_start(out=idx2_t[:], in_=idx2[:, :])
            meta_t = cpool.tile([P, NBLK * 2 * C], bf16, tag="meta")
            nc.sync.dma_start(out=meta_t[:], in_=meta[:, :])
            bcol_t = cpool.tile([P, NBLK], f32, tag="bcol")
            nc.sync.dma_start(out=bcol_t[:], in_=bcol[:, :])
            icnt_t = cpool.tile([P, N_GRAPHS], f32, tag="icnt")
            nc.sync.dma_start(out=icnt_t[:], in_=icnt[:, :])

            hloc1 = dpool.tile([PADN, HID], bf16, tag="hloc1")
            hloc2 = dpool.tile([PADN, HID], bf16, tag="hloc2")
            xnext1 = dpool.tile([NCORES, PADN, HID], bf16, tag="xn1")
            xnext2 = dpool.tile([NCORES, PADN, HID], bf16, tag="xn2")
            prdram = dpool.tile([HID, N_GRAPHS], f32, tag="prd")
            ardram = dpool.tile([HID, N_GRAPHS], f32, tag="ard")

            resid = [[residp.tile([P, P], bf16, tag=f"r{b}h{h}", name=f"r{b}h{h}")
                      for h in range(2)] for b in range(NBLK)]

            pooled_ps = [ppoolp.tile([P, N_GRAPHS], f32, tag=f"pool{h}", name=f"pool{h}")
                         for h in range(2)]

            def build_smat(b):
                """One fused DVE pass per block: S[e, j*128+t] = w'[j] * (tl[j]==t)."""
                s01 = spool.tile([P, C * P], bf16, tag="s01")
                smat = spool.tile([P, C * P], bf16, tag="smat")
                tl_ap = meta_t[:, b * 2 * C: b * 2 * C + C].unsqueeze(2) \
                    .broadcast_to([P, C, P])
                w_ap = meta_t[:, b * 2 * C + C: (b + 1) * 2 * C].unsqueeze(2) \
                    .broadcast_to([P, C, P])
                io_ap = iota_b[:, :].unsqueeze(1).broadcast_to([P, C, P])
                s01_3d = s01[:].rearrange("p (c t) -> p c t", c=C)
                smat_3d = smat[:].rearrange("p (c t) -> p c t", c=C)
                nc.vector.tensor_tensor(out=s01_3d, in0=tl_ap, in1=io_ap,
                                        op=mybir.AluOpType.is_equal)
                nc.vector.tensor_tensor(out=smat_3d, in0=s01_3d, in1=w_ap,
                                        op=mybir.AluOpType.mult)
                return smat

            def ag_full(hloc, xnext):
                nc.gpsimd.collective_compute(
                    "AllGather", bass.mybir.AluOpType.bypass,
                    replica_groups=[list(range(NCORES))],
                    ins=[hloc[:, :]], outs=[xnext[:, :, :]])

            def layer(li, tab, fdim, idx_all, wtiles, bc0, hloc, xnext=None):
                nf = fdim // P
                for b in range(NBLK):
                    xr = rpool.tile([P, C * fdim], bf16, tag="xr")
                    for j in range(C):
                        nc.gpsimd.indirect_dma_start(
                            out=xr[:, j * fdim:(j + 1) * fdim], out_offset=None,
                            in_=tab,
                            in_offset=bass.IndirectOffsetOnAxis(
                                ap=idx_all[:, b * C + j:b * C + j + 1], axis=0),
                        )
                    smat = build_smat(b)

                    aggT = [ppool.tile([P, P], f32, tag=f"agg{k}", name=f"aggps{k}")
                            for k in range(nf)]
                    for j in range(C):
                        for k in range(nf):
                            nc.tensor.matmul(
                                aggT[k][:],
                                lhsT=xr[:, j * fdim + k * P: j * fdim + (k + 1) * P],
                                rhs=smat[:, j * P:(j + 1) * P],
                                start=(j == 0), stop=(j == C - 1))

                    aggs = [wpool.tile([P, P], bf16, tag=f"aggs{k}", name=f"aggs{k}")
                            for k in range(nf)]
                    for k in range(nf):
                        nc.scalar.copy(aggs[k][:], aggT[k][:])

                    hrow = hpool.tile([P, HID], bf16, tag="hrow")
                    for h in range(2):
                        hT_ps = ppool.tile([P, P], f32, tag="ht")
                        for k in range(nf):
                            nc.tensor.matmul(
                                hT_ps[:], lhsT=wtiles[k][:, h * P:(h + 1) * P],
                                rhs=aggs[k][:], start=(k == 0), stop=(k == nf - 1))
                        hTs = wpool.tile([P, P], f32, tag=f"hTs{h}")
                        nc.scalar.activation(hTs[:], hT_ps[:], AF.Relu,
                                             bias=bias_t[:, bc0 + h:bc0 + h + 1])
                        if li == 0:
                            nc.vector.tensor_scalar(
                                out=resid[b][h][:], in0=hTs[:],
                                scalar1=tsh_t[:, bc0 + h:bc0 + h + 1], scalar2=None,
                                op0=OP.add)
                        else:
                            u = wpool.tile([P, P], bf16, tag=f"u{h}")
                            nc.vector.tensor_scalar(
                                out=u[:], in0=hTs[:],
                                scalar1=tsh_t[:, bc0 + h:bc0 + h + 1], scalar2=None,
                                op0=OP.add)
                            nc.vector.tensor_tensor(
                                out=resid[b][h][:], in0=resid[b][h][:], in1=u[:],
                                op=OP.add)
                        tp_ps = ppool.tile([P, P], bf16, tag="ht")
                        nc.tensor.transpose(tp_ps[:], resid[b][h][:], ident[:])
                        nc.scalar.copy(hrow[:, h * P:(h + 1) * P], tp_ps[:])

                    if hloc is not None:
                        nc.sync.dma_start(out=hloc[b * P:(b + 1) * P, :], in_=hrow[:])
                    else:
                        # L3: pool inline. mblk[t, g] = (batch[t]==g)
                        mblk = spool.tile([P, N_GRAPHS], bf16, tag="mblk")
                        nc.vector.tensor_tensor(
                            out=mblk[:],
                            in0=bcol_t[:, b:b + 1].broadcast_to([P, N_GRAPHS]),
                            in1=iota5_f[:], op=OP.is_equal)
                        for h in range(2):
                            nc.tensor.matmul(
                                pooled_ps[h][:], lhsT=hrow[:, h * P:(h + 1) * P],
                                rhs=mblk[:], start=(b == 0), stop=(b == NBLK - 1))

            tab1 = x_in[:, :]
            layer(0, tab1, IN_DIM, idx1_t, [w1_t], 0, hloc1, xnext1)
            ag_full(hloc1, xnext1)
            tab2 = xnext1[:, :, :].rearrange("c r f -> (c r) f")
            layer(1, tab2, HID, idx2_t, w2_t, 2, hloc2, xnext2)
            ag_full(hloc2, xnext2)
            tab3 = xnext2[:, :, :].rearrange("c r f -> (c r) f")
            layer(2, tab3, HID, idx2_t, w3_t, 4, None)

            # pooled partial sums -> DRAM -> AllReduce
            for h in range(2):
                ps = wpool.tile([P, N_GRAPHS], f32, tag=f"poolsb{h}")
                nc.vector.tensor_copy(ps[:], pooled_ps[h][:])
                nc.sync.dma_start(out=prdram[h * P:(h + 1) * P, :], in_=ps[:])
            nc.gpsimd.collective_compute(
                "AllReduce", bass.mybir.AluOpType.add,
                replica_groups=[list(range(NCORES))],
                ins=[prdram[:, :]], outs=[ardram[:, :]])

            # head: h1T[o,g] = relu(lw1.T @ (pooledT*icnt) + lb1); out = lw2.T @ h1T + lb2
            lw1_t = [cpool.tile([P, HID], f32, tag=f"lw1_{k}", name=f"lw1_{k}") for k in range(2)]
            lw2_t = cpool.tile([P, 2], f32, tag="lw2")
            lb1_t = cpool.tile([P, 2], f32, tag="lb1")
            lb2_t = cpool.tile([1, 1], f32, tag="lb2")
            for k in range(2):
                nc.sync.dma_start(out=lw1_t[k][:], in_=lw1[k * P:(k + 1) * P, :])
            nc.sync.dma_start(out=lw2_t[:], in_=lw2[:, :])
            nc.sync.dma_start(out=lb1_t[:], in_=lb1c[:, :])
            nc.sync.dma_start(out=lb2_t[:], in_=lb2c[:, :])

            par = []
            for k in range(2):
                pk = wpool.tile([P, N_GRAPHS], f32, tag=f"par{k}")
                nc.sync.dma_start(out=pk[:], in_=ardram[k * P:(k + 1) * P, :])
                pks = wpool.tile([P, N_GRAPHS], f32, tag=f"pars{k}")
                nc.vector.tensor_tensor(out=pks[:], in0=pk[:], in1=icnt_t[:], op=OP.mult)
                par.append(pks)
            h1s = []
            for h in range(2):
                h1_ps = ppool.tile([P, N_GRAPHS], f32, tag="agg0")
                for k in range(2):
                    nc.tensor.matmul(h1_ps[:], lhsT=lw1_t[k][:, h * P:(h + 1) * P],
                                     rhs=par[k][:], start=(k == 0), stop=(k == 1))
                h1sb = wpool.tile([P, N_GRAPHS], f32, tag=f"h1s{h}")
                nc.scalar.activation(h1sb[:], h1_ps[:], AF.Relu,
                                     bias=lb1_t[:, h:h + 1])
                h1s.append(h1sb)
            out_ps = ppool.tile([1, N_GRAPHS], f32, tag="agg1")
            for h in range(2):
                nc.tensor.matmul(out_ps[:], lhsT=lw2_t[:, h:h + 1],
                                 rhs=h1s[h][:], start=(h == 0), stop=(h == 1))
            out_sb = wpool.tile([1, N_GRAPHS], f32, tag="outs")
            nc.vector.tensor_scalar(out=out_sb[:], in0=out_ps[:],
                                    scalar1=lb2_t[0:1, 0:1], scalar2=None, op0=OP.add)
            nc.sync.dma_start(out=out[:, :], in_=out_sb[:])

    nc.compile()
    return nc


def _preprocess(edge_index, batch):
    """Degree-balanced node->block assignment + per-core edge lists grouped by
    target block, padded to uniform chunks."""
    src = np.asarray(edge_index[0], dtype=np.int64)
    tgt = np.asarray(edge_index[1], dtype=np.int64)
    batch = np.asarray(batch, dtype=np.int64)

    indeg = np.bincount(tgt, minlength=N_NODES).astype(np.int64)
    deg = indeg.astype(np.float64) + 1.0
    dinv = 1.0 / np.sqrt(deg)

    # balanced snake assignment of 50176 slots (incl 176 weight-0 virtual)
    slots = XROWS
    wts = np.concatenate([indeg + 1, np.zeros(slots - N_NODES, np.int64)])
    order = np.argsort(-wts, kind="stable")
    assign_block = np.empty(slots, np.int64)
    fwd = np.arange(NTOT)
    for r in range(P):
        seg = order[r * NTOT:(r + 1) * NTOT]
        assign_block[seg] = fwd if r % 2 == 0 else fwd[::-1]
    perm = np.argsort(assign_block, kind="stable")
    newpos = np.empty(slots, np.int64)
    newpos[perm] = np.arange(slots)

    allsrc = np.concatenate([src, np.arange(N_NODES, dtype=np.int64)])
    alltgt = np.concatenate([tgt, np.arange(N_NODES, dtype=np.int64)])
    allw = (dinv[allsrc] * dinv[alltgt]).astype(np.float32)

    tgt_np = newpos[alltgt]
    blkkey = tgt_np // P
    order_e = np.argsort(blkkey, kind="stable")
    allsrc, alltgt, allw = allsrc[order_e], alltgt[order_e], allw[order_e]
    tgt_np = tgt_np[order_e]

    counts = np.bincount(blkkey, minlength=NTOT)
    chunks = int(math.ceil(counts.max() / P))
    C = chunks

    GR = AGRP * P
    blk_start = np.zeros(NTOT + 1, dtype=np.int64)
    np.cumsum(counts, out=blk_start[1:])

    import ml_dtypes
    per_core = []
    for c in range(NCORES):
        idx1 = np.zeros((NBLK, P, C), dtype=np.int32)
        idx2 = np.zeros((NBLK, P, C), dtype=np.int32)
        meta = np.zeros((NBLK, P, 2 * C), dtype=np.float32)
        for b in range(NBLK):
            g = c * NBLK + b
            lo, hi = blk_start[g], blk_start[g + 1]
            n = hi - lo
            s1 = allsrc[lo:hi].astype(np.int32)
            s2 = newpos[allsrc[lo:hi]].astype(np.int32)
            tl = (tgt_np[lo:hi] - g * P).astype(np.float32)
            ww = allw[lo:hi]
            npad = C * P - n
            if npad:
                s1 = np.pad(s1, (0, npad))
                s2 = np.pad(s2, (0, npad))
                tl = np.pad(tl, (0, npad))
                ww = np.pad(ww, (0, npad))
            idx1[b] = s1.reshape(C, P).T
            idx2[b] = s2.reshape(C, P).T
            meta[b, :, :C] = tl.reshape(C, P).T
            meta[b, :, C:] = ww.reshape(C, P).T
        # batch column for pooling (pad/virtual rows -> -1)
        core_slots = perm[c * PADN:(c + 1) * PADN]   # orig ids in new order
        bvals = np.where(core_slots < N_NODES,
                         batch[np.minimum(core_slots, N_NODES - 1)], -1.0)
        bcol = bvals.reshape(NBLK, P).T.astype(np.float32)  # [P, NBLK]
        per_core.append(dict(
            idx1=idx1.transpose(1, 0, 2).reshape(P, NBLK * C).copy(),
            idx2=idx2.transpose(1, 0, 2).reshape(P, NBLK * C).copy(),
            meta=meta.transpose(1, 0, 2).reshape(P, NBLK * 2 * C)
                .astype(ml_dtypes.bfloat16),
            bcol=bcol.copy(),
        ))
    return per_core, chunks


def kernel(**inputs):
    import ml_dtypes
    from concourse.bass_utils import run_bass_kernel_spmd

    x = np.asarray(inputs["x"], dtype=np.float32)
    edge_index = np.asarray(inputs["edge_index"])
    batch = np.asarray(inputs["batch"])

    per_core, chunks = _preprocess(edge_index, batch)

    def g(k):
        return np.asarray(inputs[k], dtype=np.float32)

    params = {}
    Ws = [g("W1"), g("W2"), g("W3")]
    bs = [g("b1"), g("b2"), g("b3")]
    bias = np.zeros((P, 6), np.float32)
    tshv = np.zeros((P, 6), np.float32)
    wp = []
    for i in range(3):
        gam, be, m, v = g(f"g{i+1}"), g(f"be{i+1}"), g(f"m{i+1}"), g(f"v{i+1}")
        s = gam / np.sqrt(v + BN_EPS)
        assert (s > 0).all(), "BN scale must be positive for relu folding"
        wp.append((Ws[i] * s[None, :]).astype(ml_dtypes.bfloat16))
        bp = (bs[i] * s).astype(np.float32)
        tv = (be - m * s).astype(np.float32)
        bias[:, 2 * i] = bp[:P]
        bias[:, 2 * i + 1] = bp[P:]
        tshv[:, 2 * i] = tv[:P]
        tshv[:, 2 * i + 1] = tv[P:]
    params["w1p"], params["w2p"], params["w3p"] = wp
    params["bias"] = bias
    params["tsh"] = tshv
    params["lw1"] = g("lw1")
    lb1 = g("lb1")
    lb1c = np.zeros((P, 2), np.float32)
    lb1c[:, 0] = lb1[:P]
    lb1c[:, 1] = lb1[P:]
    params["lb1c"] = lb1c
    lw2v = g("lw2").reshape(HID)
    params["lw2"] = np.stack([lw2v[:P], lw2v[P:]], axis=1).copy()
    params["lb2c"] = g("lb2").reshape(1, 1).astype(np.float32)
    cnt = np.bincount(np.asarray(batch, dtype=np.int64), minlength=N_GRAPHS)
    icnt = (1.0 / np.maximum(cnt, 1)).astype(np.float32)
    params["icnt"] = np.tile(icnt[None, :], (P, 1))
    x_bf = x.astype(ml_dtypes.bfloat16)

    nc = _build_program(chunks)

    in_maps = []
    for c in range(NCORES):
        m = dict(params)
        m["x"] = x_bf
        m.update(per_core[c])
        in_maps.append(m)

    res = run_bass_kernel_spmd(nc, in_maps, list(range(NCORES)),
                               trace=bool(os.environ.get("GNN_TRACE")))
    if os.environ.get("GNN_TRACE"):
        print("HW exec time:", res.exec_time_ns, "ns")
    global _last_results
    _last_results = res.results
    o = res.results[0]["out"]
    return np.asarray(o, dtype=np.float32).reshape(N_GRAPHS, OUT_DIM)


# revision 13
# speedup vs baseline: 1.1429x; 1.0256x over previous
"""GCN (3x GCNConv + BN + residual, mean-pool, MLP head) on 8 trn2 NeuronCores.

Sharding: nodes are assigned to 392 blocks of 128 via degree-balanced snake
packing (equalizes per-block incident-edge counts, minimizing gather-chunk
padding); 49 blocks per core. Each core owns the edges whose TARGET lands in
its blocks (plus self-loops). GCN normalization is linear, so each layer
aggregates raw input features over incident edges (one indirect-DMA gather of
128 source rows + one PE matmul with a selection matrix per 128-edge chunk),
then applies the folded linear+BN epilogue. Activation tables are bf16;
AllGathers between layers are chunked (7 groups of 7 blocks) so they overlap
the producing layer's compute. Per-graph pooled sums are AllReduced; the tiny
MLP head runs redundantly on every core.

Device kernel per (layer, target-block of 128 nodes):
  for each 128-edge chunk: indirect-DMA gather of source rows (bf16);
  one fused DVE pass builds all selection matrices S[e,t] = w'[e]*(tl[e]==t);
  PE: aggT[f,t] += xr[:,f-chunk].T @ S_j (PSUM, bf16 inputs);
  hT[o,t] = sum_f W'[f,o].T @ agg[f,t]; ACT relu + folded bias; DVE +tsh
  (+residual); PE transpose back to [t,o] bf16 rows for the next layer's
  gather table / pooling.
"""
import math
import os
import sys

import numpy as np

sys.path.insert(0, "/opt/trn_rl_repo")

N_NODES = 50000
N_EDGES = 800000
IN_DIM = 128
HID = 256
OUT_DIM = 1
N_GRAPHS = 512
BN_EPS = 1e-5
NCORES = 8
P = 128
NBLK = 49                            # blocks per core
NTOT = NBLK * NCORES                 # 392 blocks globally
PADN = NBLK * P                      # 6272 rows per core (incl pad slots)
XROWS = PADN * NCORES                # 50176 rows in allgathered tables
AGRP = 7                             # blocks per chunked-AllGather group
NGRP = NBLK // AGRP                  # 7 groups


def _build_program(chunks):
    from concourse import bass, bacc, mybir, tile
    from concourse.masks import make_identity

    f32 = mybir.dt.float32
    bf16 = mybir.dt.bfloat16
    i32 = mybir.dt.int32
    AF = mybir.ActivationFunctionType
    OP = mybir.AluOpType

    C = chunks
    GRows = AGRP * P                 # rows per AllGather group (896)

    nc = bacc.Bacc("TRN2", target_bir_lowering=False, debug=False,
                   num_devices=NCORES)

    x_in = nc.declare_dram_parameter("x", [N_NODES, IN_DIM], bf16, isOutput=False)
    idx1 = nc.declare_dram_parameter("idx1", [P, NBLK * C], i32, isOutput=False)
    idx2 = nc.declare_dram_parameter("idx2", [P, NBLK * C], i32, isOutput=False)
    meta = nc.declare_dram_parameter("meta", [P, NBLK * 2 * C], bf16, isOutput=False)
    bcol = nc.declare_dram_parameter("bcol", [P, NBLK], f32, isOutput=False)
    w1p = nc.declare_dram_parameter("w1p", [IN_DIM, HID], bf16, isOutput=False)
    w2p = nc.declare_dram_parameter("w2p", [HID, HID], bf16, isOutput=False)
    w3p = nc.declare_dram_parameter("w3p", [HID, HID], bf16, isOutput=False)
    bias = nc.declare_dram_parameter("bias", [P, 6], f32, isOutput=False)
    tsh = nc.declare_dram_parameter("tsh", [P, 6], f32, isOutput=False)
    lw1 = nc.declare_dram_parameter("lw1", [HID, HID], f32, isOutput=False)
    lb1c = nc.declare_dram_parameter("lb1c", [P, 2], f32, isOutput=False)
    lw2 = nc.declare_dram_parameter("lw2", [P, 2], f32, isOutput=False)
    lb2c = nc.declare_dram_parameter("lb2c", [1, 1], f32, isOutput=False)
    icnt = nc.declare_dram_parameter("icnt", [P, N_GRAPHS], f32, isOutput=False)
    out = nc.declare_dram_parameter("out", [1, N_GRAPHS], f32, isOutput=True)

    with tile.TileContext(nc) as tc:
        with tc.tile_pool(name="const", bufs=1) as cpool, \
             tc.tile_pool(name="rows", bufs=6) as rpool, \
             tc.tile_pool(name="smat", bufs=2) as spool, \
             tc.tile_pool(name="work", bufs=2) as wpool, \
             tc.tile_pool(name="resid", bufs=1) as residp, \
             tc.tile_pool(name="hrow", bufs=3) as hpool, \
             tc.tile_pool(name="psum", bufs=2, space="PSUM") as ppool, \
             tc.tile_pool(name="psump", bufs=1, space="PSUM") as ppoolp, \
             tc.tile_pool(name="dram", bufs=1, space="DRAM") as dpool:

            iota_i = cpool.tile([P, P], i32, tag="ioi")
            nc.gpsimd.iota(iota_i[:], pattern=[[1, P]], base=0, channel_multiplier=0)
            iota_b = cpool.tile([P, P], bf16, tag="iob")
            nc.vector.tensor_copy(iota_b[:], iota_i[:])
            iota5_i = cpool.tile([P, N_GRAPHS], i32, tag="io5i")
            nc.gpsimd.iota(iota5_i[:], pattern=[[1, N_GRAPHS]], base=0, channel_multiplier=0)
            iota5_f = cpool.tile([P, N_GRAPHS], f32, tag="io5f")
            nc.vector.tensor_copy(iota5_f[:], iota5_i[:])
            ident = cpool.tile([P, P], bf16, tag="ident")
            make_identity(nc, ident[:])

            bias_t = cpool.tile([P, 6], f32, tag="bias")
            nc.sync.dma_start(out=bias_t[:], in_=bias[:, :])
            tsh_t = cpool.tile([P, 6], f32, tag="tsh")
            nc.sync.dma_start(out=tsh_t[:], in_=tsh[:, :])

            w1_t = cpool.tile([IN_DIM, HID], bf16, tag="w1")
            nc.sync.dma_start(out=w1_t[:], in_=w1p[:, :])
            w2_t = [cpool.tile([P, HID], bf16, tag=f"w2_{k}", name=f"w2_{k}") for k in range(2)]
            w3_t = [cpool.tile([P, HID], bf16, tag=f"w3_{k}", name=f"w3_{k}") for k in range(2)]
            for k in range(2):
                nc.sync.dma_start(out=w2_t[k][:], in_=w2p[k * P:(k + 1) * P, :])
                nc.sync.dma_start(out=w3_t[k][:], in_=w3p[k * P:(k + 1) * P, :])

            # all per-block metadata loaded upfront (small)
            idx1_t = cpool.tile([P, NBLK * C], i32, tag="idx1")
            nc.sync.dma_start(out=idx1_t[:], in_=idx1[:, :])
            idx2_t = cpool.tile([P, NBLK * C], i32, tag="idx2")
            nc.sync.dma_start(out=idx2_t[:], in_=idx2[:, :])
            meta_t = cpool.tile([P, NBLK * 2 * C], bf16, tag="meta")
            nc.sync.dma_start(out=meta_t[:], in_=meta[:, :])
            bcol_t = cpool.tile([P, NBLK], f32, tag="bcol")
            nc.sync.dma_start(out=bcol_t[:], in_=bcol[:, :])
            icnt_t = cpool.tile([P, N_GRAPHS], f32, tag="icnt")
            nc.sync.dma_start(out=icnt_t[:], in_=icnt[:, :])

            hloc1 = dpool.tile([PADN, HID], bf16, tag="hloc1")
            hloc2 = dpool.tile([PADN, HID], bf16, tag="hloc2")
            xnext1 = dpool.tile([NCORES, PADN, HID], bf16, tag="xn1",
                                addr_space="Shared")
            xnext2 = dpool.tile([NCORES, PADN, HID], bf16, tag="xn2",
                                addr_space="Shared")
            prdram = dpool.tile([HID, N_GRAPHS], f32, tag="prd")
            ardram = dpool.tile([HID, N_GRAPHS], f32, tag="ard")

            resid = [[residp.tile([P, P], bf16, tag=f"r{b}h{h}", name=f"r{b}h{h}")
                      for h in range(2)] for b in range(NBLK)]

            pooled_ps = [ppoolp.tile([P, N_GRAPHS], f32, tag=f"pool{h}", name=f"pool{h}")
                         for h in range(2)]

            def build_smat(b):
                """One fused DVE pass per block: S[e, j*128+t] = w'[j] * (tl[j]==t)."""
                s01 = spool.tile([P, C * P], bf16, tag="s01")
                smat = spool.tile([P, C * P], bf16, tag="smat")
                tl_ap = meta_t[:, b * 2 * C: b * 2 * C + C].unsqueeze(2) \
                    .broadcast_to([P, C, P])
                w_ap = meta_t[:, b * 2 * C + C: (b + 1) * 2 * C].unsqueeze(2) \
                    .broadcast_to([P, C, P])
                io_ap = iota_b[:, :].unsqueeze(1).broadcast_to([P, C, P])
                s01_3d = s01[:].rearrange("p (c t) -> p c t", c=C)
                smat_3d = smat[:].rearrange("p (c t) -> p c t", c=C)
                nc.vector.tensor_tensor(out=s01_3d, in0=tl_ap, in1=io_ap,
                                        op=mybir.AluOpType.is_equal)
                nc.vector.tensor_tensor(out=smat_3d, in0=s01_3d, in1=w_ap,
                                        op=mybir.AluOpType.mult)
                return smat

            def ag_full(hloc, xnext):
                nc.gpsimd.collective_compute(
                    "AllGather", bass.mybir.AluOpType.bypass,
                    replica_groups=[list(range(NCORES))],
                    ins=[hloc[:, :]], outs=[xnext[:, :, :]])

            def layer(li, tab, fdim, idx_all, wtiles, bc0, hloc, xnext=None):
                nf = fdim // P
                for b in range(NBLK):
                    xr = rpool.tile([P, C * fdim], bf16, tag="xr")
                    for j in range(C):
                        nc.gpsimd.indirect_dma_start(
                            out=xr[:, j * fdim:(j + 1) * fdim], out_offset=None,
                            in_=tab,
                            in_offset=bass.IndirectOffsetOnAxis(
                                ap=idx_all[:, b * C + j:b * C + j + 1], axis=0),
                        )
                    smat = build_smat(b)

                    aggT = [ppool.tile([P, P], f32, tag=f"agg{k}", name=f"aggps{k}")
                            for k in range(nf)]
                    for j in range(C):
                        for k in range(nf):
                            nc.tensor.matmul(
                                aggT[k][:],
                                lhsT=xr[:, j * fdim + k * P: j * fdim + (k + 1) * P],
                                rhs=smat[:, j * P:(j + 1) * P],
                                start=(j == 0), stop=(j == C - 1))

                    aggs = [wpool.tile([P, P], bf16, tag=f"aggs{k}", name=f"aggs{k}")
                            for k in range(nf)]
                    for k in range(nf):
                        nc.scalar.copy(aggs[k][:], aggT[k][:])

                    hrow = hpool.tile([P, HID], bf16, tag="hrow")
                    for h in range(2):
                        hT_ps = ppool.tile([P, P], f32, tag="ht")
                        for k in range(nf):
                            nc.tensor.matmul(
                                hT_ps[:], lhsT=wtiles[k][:, h * P:(h + 1) * P],
                                rhs=aggs[k][:], start=(k == 0), stop=(k == nf - 1))
                        hTs = wpool.tile([P, P], f32, tag=f"hTs{h}")
                        nc.scalar.activation(hTs[:], hT_ps[:], AF.Relu,
                                             bias=bias_t[:, bc0 + h:bc0 + h + 1])
                        if li == 0:
                            nc.vector.tensor_scalar(
                                out=resid[b][h][:], in0=hTs[:],
                                scalar1=tsh_t[:, bc0 + h:bc0 + h + 1], scalar2=None,
                                op0=OP.add)
                        else:
                            u = wpool.tile([P, P], bf16, tag=f"u{h}")
                            nc.vector.tensor_scalar(
                                out=u[:], in0=hTs[:],
                                scalar1=tsh_t[:, bc0 + h:bc0 + h + 1], scalar2=None,
                                op0=OP.add)
                            nc.vector.tensor_tensor(
                                out=resid[b][h][:], in0=resid[b][h][:], in1=u[:],
                                op=OP.add)
                        tp_ps = ppool.tile([P, P], bf16, tag="ht")
                        nc.tensor.transpose(tp_ps[:], resid[b][h][:], ident[:])
                        nc.scalar.copy(hrow[:, h * P:(h + 1) * P], tp_ps[:])

                    if hloc is not None:
                        nc.sync.dma_start(out=hloc[b * P:(b + 1) * P, :], in_=hrow[:])
                    else:
                        # L3: pool inline. mblk[t, g] = (batch[t]==g)
                        mblk = spool.tile([P, N_GRAPHS], bf16, tag="mblk")
                        nc.vector.tensor_tensor(
                            out=mblk[:],
                            in0=bcol_t[:, b:b + 1].broadcast_to([P, N_GRAPHS]),
                            in1=iota5_f[:], op=OP.is_equal)
                        for h in range(2):
                            nc.tensor.matmul(
                                pooled_ps[h][:], lhsT=hrow[:, h * P:(h + 1) * P],
                                rhs=mblk[:], start=(b == 0), stop=(b == NBLK - 1))

            tab1 = x_in[:, :]
            layer(0, tab1, IN_DIM, idx1_t, [w1_t], 0, hloc1, xnext1)
            ag_full(hloc1, xnext1)
            tab2 = xnext1[:, :, :].rearrange("c r f -> (c r) f")
            layer(1, tab2, HID, idx2_t, w2_t, 2, hloc2, xnext2)
            ag_full(hloc2, xnext2)
            tab3 = xnext2[:, :, :].rearrange("c r f -> (c r) f")
            layer(2, tab3, HID, idx2_t, w3_t, 4, None)

            # pooled partial sums -> DRAM -> AllReduce
            for h in range(2):
                ps = wpool.tile([P, N_GRAPHS], f32, tag=f"poolsb{h}")
                nc.vector.tensor_copy(ps[:], pooled_ps[h][:])
                nc.sync.dma_start(out=prdram[h * P:(h + 1) * P, :], in_=ps[:])
            nc.gpsimd.collective_compute(
                "AllReduce", bass.mybir.AluOpType.add,
                replica_groups=[list(range(NCORES))],
                ins=[prdram[:, :]], outs=[ardram[:, :]])

            # head: h1T[o,g] = relu(lw1.T @ (pooledT*icnt) + lb1); out = lw2.T @ h1T + lb2
            lw1_t = [cpool.tile([P, HID], f32, tag=f"lw1_{k}", name=f"lw1_{k}") for k in range(2)]
            lw2_t = cpool.tile([P, 2], f32, tag="lw2")
            lb1_t = cpool.tile([P, 2], f32, tag="lb1")
            lb2_t = cpool.tile([1, 1], f32, tag="lb2")
            for k in range(2):
                nc.sync.dma_start(out=lw1_t[k][:], in_=lw1[k * P:(k + 1) * P, :])
            nc.sync.dma_start(out=lw2_t[:], in_=lw2[:, :])
            nc.sync.dma_start(out=lb1_t[:], in_=lb1c[:, :])
            nc.sync.dma_start(out=lb2_t[:], in_=lb2c[:, :])

            par = []
            for k in range(2):
                pk = wpool.tile([P, N_GRAPHS], f32, tag=f"par{k}")
                nc.sync.dma_start(out=pk[:], in_=ardram[k * P:(k + 1) * P, :])
                pks = wpool.tile([P, N_GRAPHS], f32, tag=f"pars{k}")
                nc.vector.tensor_tensor(out=pks[:], in0=pk[:], in1=icnt_t[:], op=OP.mult)
                par.append(pks)
            h1s = []
            for h in range(2):
                h1_ps = ppool.tile([P, N_GRAPHS], f32, tag="agg0")
                for k in range(2):
                    nc.tensor.matmul(h1_ps[:], lhsT=lw1_t[k][:, h * P:(h + 1) * P],
                                     rhs=par[k][:], start=(k == 0), stop=(k == 1))
                h1sb = wpool.tile([P, N_GRAPHS], f32, tag=f"h1s{h}")
                nc.scalar.activation(h1sb[:], h1_ps[:], AF.Relu,
                                     bias=lb1_t[:, h:h + 1])
                h1s.append(h1sb)
            out_ps = ppool.tile([1, N_GRAPHS], f32, tag="agg1")
            for h in range(2):
                nc.tensor.matmul(out_ps[:], lhsT=lw2_t[:, h:h + 1],
                                 rhs=h1s[h][:], start=(h == 0), stop=(h == 1))
            out_sb = wpool.tile([1, N_GRAPHS], f32, tag="outs")
            nc.vector.tensor_scalar(out=out_sb[:], in0=out_ps[:],
                                    scalar1=lb2_t[0:1, 0:1], scalar2=None, op0=OP.add)
            nc.sync.dma_start(out=out[:, :], in_=out_sb[:])

    nc.compile()
    return nc


def _preprocess(edge_index, batch):
    """Degree-balanced node->block assignment + per-core edge lists grouped by
    target block, padded to uniform chunks."""
    src = np.asarray(edge_index[0], dtype=np.int64)
    tgt = np.asarray(edge_index[1], dtype=np.int64)
    batch = np.asarray(batch, dtype=np.int64)

    indeg = np.bincount(tgt, minlength=N_NODES).astype(np.int64)
    deg = indeg.astype(np.float64) + 1.0
    dinv = 1.0 / np.sqrt(deg)

    # balanced snake assignment of 50176 slots (incl 176 weight-0 virtual)
    slots = XROWS
    wts = np.concatenate([indeg + 1, np.zeros(slots - N_NODES, np.int64)])
    order = np.argsort(-wts, kind="stable")
    assign_block = np.empty(slots, np.int64)
    fwd = np.arange(NTOT)
    for r in range(P):
        seg = order[r * NTOT:(r + 1) * NTOT]
        assign_block[seg] = fwd if r % 2 == 0 else fwd[::-1]
    perm = np.argsort(assign_block, kind="stable")
    newpos = np.empty(slots, np.int64)
    newpos[perm] = np.arange(slots)

    allsrc = np.concatenate([src, np.arange(N_NODES, dtype=np.int64)])
    alltgt = np.concatenate([tgt, np.arange(N_NODES, dtype=np.int64)])
    allw = (dinv[allsrc] * dinv[alltgt]).astype(np.float32)

    tgt_np = newpos[alltgt]
    blkkey = tgt_np // P
    order_e = np.argsort(blkkey, kind="stable")
    allsrc, alltgt, allw = allsrc[order_e], alltgt[order_e], allw[order_e]
    tgt_np = tgt_np[order_e]

    counts = np.bincount(blkkey, minlength=NTOT)
    chunks = int(math.ceil(counts.max() / P))
    C = chunks

    GR = AGRP * P
    blk_start = np.zeros(NTOT + 1, dtype=np.int64)
    np.cumsum(counts, out=blk_start[1:])

    import ml_dtypes
    per_core = []
    for c in range(NCORES):
        idx1 = np.zeros((NBLK, P, C), dtype=np.int32)
        idx2 = np.zeros((NBLK, P, C), dtype=np.int32)
        meta = np.zeros((NBLK, P, 2 * C), dtype=np.float32)
        for b in range(NBLK):
            g = c * NBLK + b
            lo, hi = blk_start[g], blk_start[g + 1]
            n = hi - lo
            s1 = allsrc[lo:hi].astype(np.int32)
            s2 = newpos[allsrc[lo:hi]].astype(np.int32)
            tl = (tgt_np[lo:hi] - g * P).astype(np.float32)
            ww = allw[lo:hi]
            npad = C * P - n
            if npad:
                s1 = np.pad(s1, (0, npad))
                s2 = np.pad(s2, (0, npad))
                tl = np.pad(tl, (0, npad))
                ww = np.pad(ww, (0, npad))
            idx1[b] = s1.reshape(C, P).T
            idx2[b] = s2.reshape(C, P).T
            meta[b, :, :C] = tl.reshape(C, P).T
            meta[b, :, C:] = ww.reshape(C, P).T
        # batch column for pooling (pad/virtual rows -> -1)
        core_slots = perm[c * PADN:(c + 1) * PADN]   # orig ids in new order
        bvals = np.where(core_slots < N_NODES,
                         batch[np.minimum(core_slots, N_NODES - 1)], -1.0)
        bcol = bvals.reshape(NBLK, P).T.astype(np.float32)  # [P, NBLK]
        per_core.append(dict(
            idx1=idx1.transpose(1, 0, 2).reshape(P, NBLK * C).copy(),
            idx2=idx2.transpose(1, 0, 2).reshape(P, NBLK * C).copy(),
            meta=meta.transpose(1, 0, 2).reshape(P, NBLK * 2 * C)
                .astype(ml_dtypes.bfloat16),
            bcol=bcol.copy(),
        ))
    return per_core, chunks


def kernel(**inputs):
    import ml_dtypes
    from concourse.bass_utils import run_bass_kernel_spmd

    x = np.asarray(inputs["x"], dtype=np.float32)
    edge_index = np.asarray(inputs["edge_index"])
    batch = np.asarray(inputs["batch"])

    per_core, chunks = _preprocess(edge_index, batch)

    def g(k):
        return np.asarray(inputs[k], dtype=np.float32)

    params = {}
    Ws = [g("W1"), g("W2"), g("W3")]
    bs = [g("b1"), g("b2"), g("b3")]
    bias = np.zeros((P, 6), np.float32)
    tshv = np.zeros((P, 6), np.float32)
    wp = []
    for i in range(3):
        gam, be, m, v = g(f"g{i+1}"), g(f"be{i+1}"), g(f"m{i+1}"), g(f"v{i+1}")
        s = gam / np.sqrt(v + BN_EPS)
        assert (s > 0).all(), "BN scale must be positive for relu folding"
        wp.append((Ws[i] * s[None, :]).astype(ml_dtypes.bfloat16))
        bp = (bs[i] * s).astype(np.float32)
        tv = (be - m * s).astype(np.float32)
        bias[:, 2 * i] = bp[:P]
        bias[:, 2 * i + 1] = bp[P:]
        tshv[:, 2 * i] = tv[:P]
        tshv[:, 2 * i + 1] = tv[P:]
    params["w1p"], params["w2p"], params["w3p"] = wp
    params["bias"] = bias
    params["tsh"] = tshv
    params["lw1"] = g("lw1")
    lb1 = g("lb1")
    lb1c = np.zeros((P, 2), np.float32)
    lb1c[:, 0] = lb1[:P]
    lb1c[:, 1] = lb1[P:]
    params["lb1c"] = lb1c
    lw2v = g("lw2").reshape(HID)
    params["lw2"] = np.stack([lw2v[:P], lw2v[P:]], axis=1).copy()
    params["lb2c"] = g("lb2").reshape(1, 1).astype(np.float32)
    cnt = np.bincount(np.asarray(batch, dtype=np.int64), minlength=N_GRAPHS)
    icnt = (1.0 / np.maximum(cnt, 1)).astype(np.float32)
    params["icnt"] = np.tile(icnt[None, :], (P, 1))
    x_bf = x.astype(ml_dtypes.bfloat16)

    nc = _build_program(chunks)

    in_maps = []
    for c in range(NCORES):
        m = dict(params)
        m["x"] = x_bf
        m.update(per_core[c])
        in_maps.append(m)

    res = run_bass_kernel_spmd(nc, in_maps, list(range(NCORES)),
                               trace=bool(os.environ.get("GNN_TRACE")))
    if os.environ.get("GNN_TRACE"):
        print("HW exec time:", res.exec_time_ns, "ns")
    global _last_results
    _last_results = res.results
    o = res.results[0]["out"]
    return np.asarray(o, dtype=np.float32).reshape(N_GRAPHS, OUT_DIM)
